# revision 58
# baseline (speedup 1.0000x reference)
"""Multi-head causal attention with RoPE on 8 TRN2 NeuronCores.

Sharding: batch (2) x head-groups (4 of 4 heads) -> 8 cores.
v2: Q/K projections run as fp8-e4m3 DoubleRow matmuls (2x PE throughput on
the K=1024 contraction; host pre-casts x and 32-scaled W_q/W_k to fp8, the
1024x score growth is folded into the softmax exp scale). V projection,
scores, PV and W_o stay bf16 (error budget: fp8 on those overflows the 2e-2
gate; scores/PV are PSUM-write-port bound at Dh=64 so fp8 wouldn't pay
anyway). ACT keeps only exp + O drains; PSUM->SBUF copies (RoPE staging,
V staging, W_o staging) move to the otherwise-idle GpSimd, and the softmax
reciprocal moves to DVE, so the exp stream owns the ACT engine.
Per-core structure otherwise as v1: per 512-row s-chunk, transposed scores
S^T = Kr @ Qr^T with both heads of a pair in one [128,1024] PSUM tile,
causal block-skip + column-subrange matmuls on diagonal blocks, PV matmul
with a ones-column on V accumulating the softmax denominator, ones-matmul
broadcast of the reciprocal row, deferred normalize chain, W_o a half-chunk
behind. Host sums the 4 per-batch partials.
"""
import os
import sys

sys.path.insert(0, "/opt/trn_rl_repo")

import ml_dtypes
import numpy as np

import concourse.bass as bass
import concourse.mybir as mybir
import concourse.tile as tile
from concourse import bass_utils

F32 = mybir.dt.float32
BF16 = mybir.dt.bfloat16
F8 = mybir.dt.float8e4
F32R = mybir.dt.float32r

DT = BF16
DT_NP = ml_dtypes.bfloat16
F8_NP = ml_dtypes.float8_e4m3fn

B, S, E, H, Dh = 2, 2048, 1024, 16, 64
HG = 4            # heads per core
HD = HG * Dh      # 256 output channels per core
SCALE = float(1.0 / np.sqrt(np.float32(1024.0)))
WS = 32.0         # host scale on W_q/W_k so fp8 sees ~unit-variance weights
SCALE_QK = SCALE / (WS * WS)
ROPE_BASE = 10000.0
NCHUNK = S // 512     # 4 s-chunks of 512
NTB = S // 128        # 16 t-blocks of 128
SHUF16 = list(range(16, 32)) + list(range(0, 16))

Exp = mybir.ActivationFunctionType.Exp
Ln = mybir.ActivationFunctionType.Ln
MUL = mybir.AluOpType.mult
ADD = mybir.AluOpType.add
DR = mybir.MatmulPerfMode.DoubleRow


def _build_program(legalize=True):
    nc = bass.Bass("TRN2", target_bir_lowering=False, debug=False)

    xT = nc.dram_tensor("xT", [128, NCHUNK, 8, 512], DT, kind="ExternalInput")
    xT8 = nc.dram_tensor("xT8", [128, NCHUNK, 8, 512], F8, kind="ExternalInput")
    wq8 = nc.dram_tensor("wq8", [128, 8, HD], F8, kind="ExternalInput")
    wk8 = nc.dram_tensor("wk8", [128, 8, HD], F8, kind="ExternalInput")
    wv = nc.dram_tensor("wv", [128, 8, HD], DT, kind="ExternalInput")
    wo = nc.dram_tensor("wo", [128, 2, E], DT, kind="ExternalInput")
    cosd = nc.dram_tensor("cosd", [128, S], DT, kind="ExternalInput")
    sins = nc.dram_tensor("sins", [128, S], DT, kind="ExternalInput")
    trim = nc.dram_tensor("trim", [128, 128], DT, kind="ExternalInput")
    sel2c = nc.dram_tensor("sel2c", [33, 128], F32R, kind="ExternalInput")
    lnz = nc.dram_tensor("lnz", [33, 512], F32R, kind="ExternalInput")
    y = nc.dram_tensor("y", [S, E], DT, kind="ExternalOutput")

    with tile.TileContext(nc) as tc:
        with (
            tc.tile_pool(name="persist", bufs=1) as pp,
            tc.tile_pool(name="xchunks", bufs=3) as xp,
            tc.tile_pool(name="x8chunks", bufs=3) as xp8,
            tc.tile_pool(name="ropetmp", bufs=2) as rt,
            tc.tile_pool(name="att_es", bufs=6) as ep,
            tc.tile_pool(name="att_row", bufs=2) as rp,
            tc.tile_pool(name="osb", bufs=4) as op_,
            tc.tile_pool(name="ystg", bufs=2) as yp,
            tc.tile_pool(name="ps_big", bufs=3, space="PSUM") as psB,
            tc.tile_pool(name="ps_ot", bufs=2, space="PSUM") as psO,
        ):
            # ---- persistent tensors ----
            # Qr^T zero-padded per head half: qz[:, hi, blk, s] has rows of
            # head 2*blk+hi live and the other 64 rows zero, so the score
            # contraction runs over the full 128 partitions.
            qz = pp.tile([128, 2, 2, S], DT)
            krt = pp.tile([128, 2, S], DT)   # Kr^T
            vau = pp.tile([128, NTB, HG, 65], DT)  # V + ones col per (tb, h)
            ot = pp.tile([128, 2, S], DT)    # O^T normalized

            # stationary for PE-warming matmuls
            hW = pp.tile([128, 128], DT)
            nc.vector.memset(hW[:], 1.0)
            wrm = pp.tile([1, 16], DT)
            # single shared ln-rows tile: its write (pi2) / broadcast-read
            # (pi6) windows alternate without overlap; zero rows once so the
            # selector matmul never reads uninitialized memory
            lnr_sh = pp.tile([33, 512], F32R)

            def heat(n=10):
                # full-array 128x128 matmuls to trip the HAM activity window
                # back to K=8/8. Scratch lands in a big-ring PSUM slot whose
                # next real matmul uses start=True and overwrites it.
                htile = psB.tile([128, 128], F32, tag="big", name="heat")
                for _ in range(n):
                    nc.tensor.matmul(htile[:], hW[:], hW[:],
                                     start=True, stop=True)

            # Initial loads fan out over both hardware DMA queues so the
            # first-needed tensors don't wait behind the rest.
            # ordered by first use: Q-proj inputs, RoPE tables, K, then
            # the attention-phase tensors (V/x bf16, masks, W_o)
            wq8_sb = pp.tile([128, 8, HD], F8)
            nc.sync.dma_start(wq8_sb[:], wq8.ap())
            xc80 = xp8.tile([128, 8, 512], F8, tag="xc8", name="xc80")
            nc.scalar.dma_start(xc80[:], xT8.ap()[:, 0, :, :])
            cos_sb = pp.tile([128, S], DT)
            nc.sync.dma_start(cos_sb[:], cosd.ap())
            sin_sb = pp.tile([128, S], DT)
            nc.scalar.dma_start(sin_sb[:], sins.ap())
            wk8_sb = pp.tile([128, 8, HD], F8)
            nc.sync.dma_start(wk8_sb[:], wk8.ap())
            wv_sb = pp.tile([128, 8, HD], DT)
            nc.scalar.dma_start(wv_sb[:], wv.ap())
            xc0 = xp.tile([128, 8, 512], DT, tag="xc", name="xc0")
            nc.sync.dma_start(xc0[:, 0:4, :], xT.ap()[:, 0, 0:4, :])
            nc.scalar.dma_start(xc0[:, 4:8, :], xT.ap()[:, 0, 4:8, :])
            tri_sb = pp.tile([128, 128], DT)
            nc.scalar.dma_start(tri_sb[:], trim.ap())
            sel2_sb = pp.tile([33, 128], F32R)
            nc.scalar.dma_start(sel2_sb[:], sel2c.ap())
            wo_sb = pp.tile([128, 2, E], DT)
            nc.scalar.dma_start(wo_sb[:], wo.ap())

            nc.scalar.dma_start(lnr_sh[:], lnz.ap())
            nc.gpsimd.memset(qz[64:128, 0, :, :], 0.0)
            nc.gpsimd.memset(qz[0:64, 1, :, :], 0.0)
            nc.gpsimd.memset(vau[:, :, :, 64:65], 1.0)

            # preload the ACT exp/ln tables while DMAs stream (first real
            # activation otherwise eats a 1.3us ACT_TABLE_LOAD); writes to a
            # dedicated scratch so nothing waits on the slow first GpSimd op
            with nc.allow_low_precision(reason="ACT table warm"):
                nc.scalar.activation(wrm[:], hW[0:1, 0:16], Exp, bias=0.0, scale=1.0)
                nc.scalar.activation(wrm[:], hW[0:1, 0:16], Ln, bias=0.0, scale=1.0)

            # warm the PE during the initial DMA streams
            heat(n=58)

            def emit_ln(osb, lnr):
                # ln of the two softmax denominator rows into adjacent
                # partitions of one lnr tile (from the drained osb rows)
                for hi in range(2):
                    nc.scalar.activation(lnr[32 * hi:32 * hi + 1, :],
                                         osb[hi][64:65, :], Ln, bias=0.0, scale=1.0)

            def emit_ln_psum(otp, lnr):
                # tail path: ln straight off the PSUM accumulator rows
                for hi in range(2):
                    nc.scalar.activation(lnr[32 * hi:32 * hi + 1, :],
                                         otp[hi][64:65, :], Ln, bias=0.0, scale=1.0)

            def emit_bcmult(sc, hp, osb, lnr):
                # broadcast the two ln rows with one selector matmul, one
                # full-width exp(-x) recovers the reciprocals, then DVE
                # scales the O rows into ot. Emitted several blocks after
                # emit_ln so the bc matmul never head-of-line-blocks the
                # in-order PE queue.
                ss = slice(sc * 512, (sc + 1) * 512)
                bc = psB.tile([128, 512], F32, tag="big", name="bc")
                nc.tensor.matmul(bc[:], sel2_sb[:], lnr[:], start=True, stop=True)
                ew = rp.tile([64, 2, 512], DT, tag="ew", name="ew")
                with nc.allow_low_precision(reason="recip broadcast"):
                    for hi in range(2):
                        nc.scalar.activation(ew[:, hi, :],
                                             bc[hi * 64:(hi + 1) * 64, :],
                                             Exp, bias=0.0, scale=-1.0)
                for hi in range(2):
                    with nc.allow_low_precision(reason="normalized O rows"):
                        nc.vector.tensor_tensor(ot[hi * 64:(hi + 1) * 64, hp, ss],
                                                osb[hi][0:64, :],
                                                ew[:, hi, :], MUL)

            def emit_wo_sbl(sc, sbl, tail_heat=False):
                # W_o partials for one 128-row s-block of chunk sc; both
                # 512-col E halves accumulate in one [128,1024] PSUM pair.
                sb_i = sc * 4 + sbl
                tsl = slice(sb_i * 128, (sb_i + 1) * 128)
                py = psB.tile([128, 1024], F32, tag="big", name="py")
                for ec in range(2):
                    for blk in range(2):
                        nc.tensor.matmul(
                            py[:, ec * 512:(ec + 1) * 512], ot[:, blk, tsl],
                            wo_sb[:, blk, ec * 512:(ec + 1) * 512],
                            start=(blk == 0), stop=(blk == 1),
                        )
                ystg = yp.tile([128, E], DT, tag="y")
                with nc.allow_low_precision(reason="partial sum staging"):
                    if tail_heat:
                        # tail: DVE runs the normalize mults; stage on ACT
                        nc.scalar.copy(ystg[:], py[:])
                        heat(n=2)
                    else:
                        nc.vector.tensor_copy(ystg[:], py[:])
                nc.sync.dma_start(y.ap()[tsl, :], ystg[:])

            def emit_wo(sc):
                for sbl in range(4):
                    emit_wo_sbl(sc, sbl, tail_heat=(sc == 3))

            pend_norm = None  # (sc, osb) for hp=1, normalized next chunk
            osb_prev = {}

            def dma_chunk(sc):
                xc = xp.tile([128, 8, 512], DT, tag="xc", name=f"xc{sc}")
                nc.sync.dma_start(xc[:], xT.ap()[:, sc, :, :])
                xc8 = xp8.tile([128, 8, 512], F8, tag="xc8", name=f"xc8{sc}")
                nc.scalar.dma_start(xc8[:], xT8.ap()[:, sc, :, :])
                return xc, xc8

            def vproj(sc, xc, tbl):
                # V rows for one of chunk sc's 4 new t-blocks; only the
                # diagonal PVs at the end of this chunk's tb loop need them.
                tb = sc * 4 + tbl
                pv = psB.tile([128, 256], F32, tag="big", name="pv")
                for e in range(8):
                    nc.tensor.matmul(
                        pv[:], xc[:, e, tbl * 128:(tbl + 1) * 128],
                        wv_sb[:, e, :], start=(e == 0), stop=(e == 7),
                    )
                with nc.allow_low_precision(reason="rounded matmul input"):
                    nc.vector.tensor_copy(
                        vau[:, tb, :, 0:64],
                        pv[:].rearrange("p (h d) -> p h d", d=64),
                    )

            def qkproj_mb(sc, xc8, w_sb, dst, mb):
                # one 128-chan half of a fp8 DoubleRow projection + its RoPE
                # for chunk sc; dst=None -> qz (zero-padded halves), else
                # krt. mb0 feeds blk0 (hp0) scores, mb1 feeds hp1 -- so mb1
                # pieces can be emitted much later.
                ss = slice(sc * 512, (sc + 1) * 512)
                pq = psB.tile([128, 512], F32, tag="big", name="pq")
                for ktp in range(4):
                    nc.tensor.matmul(
                        pq[:],
                        w_sb[:, 2 * ktp:2 * ktp + 2, mb * 128:(mb + 1) * 128],
                        xc8[:, 2 * ktp:2 * ktp + 2, :],
                        start=(ktp == 0), stop=(ktp == 3),
                        perf_mode=DR,
                    )
                a = rt.tile([128, 512], DT, tag="a")
                with nc.allow_low_precision(reason="rounded matmul input"):
                    nc.vector.tensor_copy(a[:], pq[:])
                bsh = rt.tile([128, 512], DT, tag="b")
                nc.vector.stream_shuffle(bsh[:], a[:], SHUF16)
                t1 = rt.tile([128, 512], DT, tag="t1")
                t2 = rt.tile([128, 512], DT, tag="t2")
                with nc.allow_low_precision(reason="rounded matmul input"):
                    nc.vector.tensor_tensor(t1[:], bsh[:], sin_sb[:, ss], MUL)
                    nc.vector.tensor_tensor(t2[:], a[:], cos_sb[:, ss], MUL)
                    if dst is None:  # Q: split into zero-padded halves
                        nc.vector.tensor_tensor(
                            qz[0:64, 0, mb, ss], t2[0:64, :], t1[0:64, :], ADD)
                        nc.vector.tensor_tensor(
                            qz[64:128, 1, mb, ss], t2[64:128, :], t1[64:128, :], ADD)
                    else:
                        nc.vector.tensor_tensor(
                            dst[:, mb, ss], t2[:, :], t1[:, :], ADD)

            # ---- chunk 0 prologue: mb0 halves first so hp0 scores can
            # start while the mb1 halves are still in flight.
            xcs = {0: (xc0, xc80)}
            qkproj_mb(0, xc80, wq8_sb, None, 0)
            qkproj_mb(0, xc80, wk8_sb, krt, 0)
            # chunk 0's V projection runs here, filling the PE while the DVE
            # RoPE chains for the mb0 halves drain; its vau tiles are only
            # needed once attention starts.
            for tbl in range(4):
                vproj(0, xc0, tbl)
            qkproj_mb(0, xc80, wq8_sb, None, 1)
            qkproj_mb(0, xc80, wk8_sb, krt, 1)


            for sc in range(NCHUNK):
                ntb = 4 * sc + 4
                xc, xc8 = xcs[sc]

                # ---- attention for this chunk, one head-pair at a time,
                # with next-chunk projection, V, W_o and normalize chains
                # woven between attention blocks so no engine sees a burst.
                osb_h = [None, None]
                fulls = list(range(4 * sc))
                diags = [4 * sc + i for i in range(4)]
                if sc == 0:
                    order = diags
                else:
                    k = len(fulls) // 4
                    order = []
                    fi = 0
                    for i, d in enumerate(diags):
                        take = max(2, k) if i == 0 else k
                        take = min(take, len(fulls) - fi)
                        order += fulls[fi:fi + take]
                        fi += take
                        order.append(d)
                    order += fulls[fi:]
                qk_step = max(ntb // 4, 1)

                for hp in range(2):
                    otp = [psO.tile([65, 512], F32, tag="ot", name=f"otp{hi}")
                           for hi in range(2)]
                    blk = hp
                    wo_step = max(ntb // 4, 2)
                    pv_prev = None  # (pi, tb, lo, es)
                    for pi, tb in enumerate(order):
                        m = tb - 4 * sc
                        lo = 128 * max(m, 0)  # diag: skip cols left of block
                        if hp == 0:
                            if pi == 0 and sc < 3:
                                xcs[sc + 1] = dma_chunk(sc + 1)
                            if pi < 4 and sc > 0:
                                vproj(sc, xc, pi)
                            if pi == 1 and sc == 1:
                                # chunk 0 hp0's bc didn't fit in its 4-block
                                # hp1 loop; runs here
                                emit_bcmult(0, 0, *osb_prev[(0, 0)])
                            if pi == 2 and pend_norm is not None:
                                emit_ln(pend_norm[1][0], pend_norm[1][1])
                            if pi == 6 and pend_norm is not None:
                                emit_bcmult(pend_norm[0], 1, *pend_norm[1])
                                pend_norm = None
                        else:
                            if pi == 2:
                                # deferred normalize of this chunk's hp=0
                                emit_ln(osb_h[0][0], osb_h[0][1])
                            if pi == 6:
                                emit_bcmult(sc, 0, *osb_h[0])
                            if (sc > 0 and pi >= 1
                                    and (pi - 1) % wo_step == 0
                                    and (pi - 1) // wo_step < 4):
                                # W_o of the previous chunk, one s-block at
                                # a time
                                emit_wo_sbl(sc - 1, (pi - 1) // wo_step)
                            if sc < 3 and pi % qk_step == 0 and pi // qk_step < 4:
                                # next chunk's projection, one (proj, mb)
                                # piece at a time: Q-mb0, K-mb0, Q-mb1, K-mb1
                                j = pi // qk_step
                                w_sb_n = (wq8_sb, wk8_sb)[j % 2]
                                dst_n = (None, krt)[j % 2]
                                qkproj_mb(sc + 1, xcs[sc + 1][1], w_sb_n,
                                          dst_n, j // 2)
                        pss = psB.tile([128, 1024], F32, tag="big", name="pss")
                        ps3 = pss[:].rearrange("p (h s) -> p h s", h=2)
                        for hi in range(2):
                            nc.tensor.matmul(
                                ps3[:, hi, lo:512],
                                krt[:, blk, tb * 128:(tb + 1) * 128],
                                qz[:, hi, blk, sc * 512 + lo:(sc + 1) * 512],
                                start=True, stop=True,
                            )
                        es = ep.tile([128, 1024], DT, tag="es", name="es")
                        es3 = es[:].rearrange("p (h s) -> p h s", h=2)
                        with nc.allow_low_precision(reason="rounded matmul input"):
                            nc.scalar.activation(es3[:, :, lo:512], ps3[:, :, lo:512],
                                                 Exp, bias=0.0, scale=SCALE_QK)
                        if m >= 0:  # mask the diagonal 128-col triangle
                            trib = tri_sb[:].rearrange("p (o s) -> p o s", o=1).to_broadcast((128, 2, 128))
                            with nc.allow_low_precision(reason="rounded matmul input"):
                                nc.vector.tensor_tensor(
                                    es3[:, :, lo:lo + 128], es3[:, :, lo:lo + 128],
                                    trib, MUL)
                        # PV trails the score stream by one block: the PE
                        # always has the next scores queued ahead of a PV
                        # that may still be waiting on its exp.
                        if pv_prev is not None:
                            ppi, ptb, plo, pes = pv_prev
                            for hi in range(2):
                                nc.tensor.matmul(
                                    otp[hi][:, plo:512],
                                    vau[:, ptb, 2 * hp + hi, :],
                                    pes[:, hi * 512 + plo:(hi + 1) * 512],
                                    start=(ppi == 0), stop=False,
                                    skip_group_check=True,
                                )
                        pv_prev = (pi, tb, lo, es)
                    ppi, ptb, plo, pes = pv_prev
                    for hi in range(2):
                        nc.tensor.matmul(
                            otp[hi][:, plo:512], vau[:, ptb, 2 * hp + hi, :],
                            pes[:, hi * 512 + plo:(hi + 1) * 512],
                            start=(ppi == 0), stop=True,
                            skip_group_check=True,
                        )
                    if sc == 3 and hp == 1:
                        # tail: start the ln chain straight off PSUM so it
                        # overlaps the accumulator drain
                        emit_ln_psum(otp, lnr_sh)
                    # drain O accumulators to SBUF promptly so the two PSUM
                    # banks recycle for the next head-pair; the two copies
                    # run on different engines so the drain latency halves.
                    osb = [op_.tile([65, 512], DT, tag="osb", name=f"osb{hi}")
                           for hi in range(2)]
                    with nc.allow_low_precision(reason="pre-normalize O"):
                        nc.vector.tensor_copy(osb[0][:], otp[0][:])
                        nc.scalar.copy(osb[1][:], otp[1][:])
                    osb_h[hp] = (osb, lnr_sh)

                osb_prev[(sc, 0)] = osb_h[0]
                if sc == 3:
                    # ln already emitted off PSUM at the drain above; heats
                    # keep the PE fed while the ACT chain completes.
                    heat(n=32)
                    emit_bcmult(3, 1, osb_h[1][0], lnr_sh)
                    emit_wo(3)
                else:
                    pend_norm = (sc, osb_h[1])

    if legalize:
        _legalize_waits(nc)
    return nc


def _legalize_waits(nc, max_waits=1):
    """Split >max_waits sync waits onto preceding same-engine NoOps
    (several instruction encodings only have one sync-wait slot)."""
    for fn in nc.m.functions:
        for bb in fn.blocks:
            new_insts = []
            for inst in bb.instructions:
                si = inst.sync_info
                waits = list(si.on_wait) if si is not None and si.on_wait else []
                if len(waits) > max_waits:
                    carry, keep = waits[:-max_waits], waits[-max_waits:]
                    for i, w in enumerate(carry):
                        new_insts.append(mybir.InstNoOp(
                            name=f"{inst.name}_wsplit{i}",
                            engine=inst.engine,
                            bass_nofuse=True,
                            sync_info=mybir.SyncInfo(on_wait=[w], on_update=[]),
                        ))
                    si.on_wait = keep
                new_insts.append(inst)
            bb.instructions[:] = new_insts


def _host_constants():
    # RoPE channel permutation: row r (within a head, 0..63) holds source
    # channel d = 2*i + odd with i = 16*(r//32) + r%16, odd = (r%32)//16.
    r = np.arange(64)
    i_ = 16 * (r // 32) + (r % 16)
    odd = (r % 32) // 16
    dsrc = 2 * i_ + odd  # source channel per permuted row

    inv_freq = ROPE_BASE ** (-(i_.astype(np.float64)) * 2.0 / Dh)
    ang = np.arange(S, dtype=np.float64)[None, :] * inv_freq[:, None]  # [64, S]
    cos64 = np.cos(ang)
    sin64 = np.sin(ang) * np.where(odd == 0, -1.0, 1.0)[:, None]
    cosd = np.tile(cos64, (2, 1)).astype(DT_NP)
    sins = np.tile(sin64, (2, 1)).astype(DT_NP)

    t = np.arange(128)[:, None]
    s = np.arange(128)[None, :]
    trim = (t <= s).astype(DT_NP)

    sel2 = np.zeros((33, 128), np.float32)
    sel2[0, 0:64] = 1
    sel2[32, 64:128] = 1
    return dsrc, cosd, sins, trim, sel2


def _wlay(w, dt=None):  # [E, HD] -> [p, ko, m] contiguous
    return np.ascontiguousarray(w.reshape(8, 128, HD).transpose(1, 0, 2)).astype(dt or DT_NP)


def _wolay(w):  # [HD, E] -> [p, ko, e] contiguous
    return np.ascontiguousarray(w.reshape(2, 128, E).transpose(1, 0, 2)).astype(DT_NP)


_CACHE = {}


def _run(inputs, trace=False):
    if "nc" not in _CACHE:
        _CACHE["nc"] = _build_program()
        _CACHE["consts"] = _host_constants()
    nc = _CACHE["nc"]
    dsrc, cosd, sins, trim, sel2 = _CACHE["consts"]

    x = np.ascontiguousarray(np.asarray(inputs["x"]), dtype=np.float32)
    W_q = np.asarray(inputs["W_q"], dtype=np.float32)
    W_k = np.asarray(inputs["W_k"], dtype=np.float32)
    W_v = np.asarray(inputs["W_v"], dtype=np.float32)
    W_o = np.asarray(inputs["W_o"], dtype=np.float32)

    # [p, sc, eo, s] so each chunk DMA is contiguous per partition
    xTc = [np.ascontiguousarray(
        x[b].reshape(NCHUNK, 512, 8, 128).transpose(3, 0, 2, 1))
        for b in range(B)]
    xTb = [t.astype(DT_NP) for t in xTc]
    xT8 = [t.astype(F8_NP) for t in xTc]

    in_maps = []
    for c in range(8):
        b, g = divmod(c, 4)
        heads = np.arange(4 * g, 4 * g + 4)
        rows_qk = (heads[:, None] * 64 + dsrc[None, :]).reshape(-1)   # permuted
        rows_v = (heads[:, None] * 64 + np.arange(64)[None, :]).reshape(-1)
        in_maps.append({
            "xT": xTb[b],
            "xT8": xT8[b],
            "wq8": _wlay(W_q[rows_qk].T * WS, F8_NP),
            "wk8": _wlay(W_k[rows_qk].T * WS, F8_NP),
            "wv": _wlay(W_v[rows_v].T),
            "wo": _wolay(W_o[:, rows_v].T),
            "cosd": cosd, "sins": sins, "trim": trim, "sel2c": sel2,
            "lnz": np.zeros((33, 512), np.float32),
        })

    res = bass_utils.run_bass_kernel_spmd(
        nc, in_maps, core_ids=list(range(8)), trace=trace,
    )
    out = np.zeros((B, S, E), np.float32)
    for c in range(8):
        out[c // 4] += res.results[c]["y"].astype(np.float32)
    return out, res


def kernel(**inputs):
    out, _ = _run(inputs, trace=False)
    return out


# revision 59
# speedup vs baseline: 1.0043x; 1.0043x over previous
"""Multi-head causal attention with RoPE on 8 TRN2 NeuronCores.

Sharding: batch (2) x head-groups (4 of 4 heads) -> 8 cores.
v2: Q/K projections run as fp8-e4m3 DoubleRow matmuls (2x PE throughput on
the K=1024 contraction; host pre-casts x and 32-scaled W_q/W_k to fp8, the
1024x score growth is folded into the softmax exp scale). V projection,
scores, PV and W_o stay bf16 (error budget: fp8 on those overflows the 2e-2
gate; scores/PV are PSUM-write-port bound at Dh=64 so fp8 wouldn't pay
anyway). ACT keeps only exp + O drains; PSUM->SBUF copies (RoPE staging,
V staging, W_o staging) move to the otherwise-idle GpSimd, and the softmax
reciprocal moves to DVE, so the exp stream owns the ACT engine.
Per-core structure otherwise as v1: per 512-row s-chunk, transposed scores
S^T = Kr @ Qr^T with both heads of a pair in one [128,1024] PSUM tile,
causal block-skip + column-subrange matmuls on diagonal blocks, PV matmul
with a ones-column on V accumulating the softmax denominator, ones-matmul
broadcast of the reciprocal row, deferred normalize chain, W_o a half-chunk
behind. Host sums the 4 per-batch partials.
"""
import os
import sys

sys.path.insert(0, "/opt/trn_rl_repo")

import ml_dtypes
import numpy as np

import concourse.bass as bass
import concourse.mybir as mybir
import concourse.tile as tile
from concourse import bass_utils

F32 = mybir.dt.float32
BF16 = mybir.dt.bfloat16
F8 = mybir.dt.float8e4
F32R = mybir.dt.float32r

DT = BF16
DT_NP = ml_dtypes.bfloat16
F8_NP = ml_dtypes.float8_e4m3fn

B, S, E, H, Dh = 2, 2048, 1024, 16, 64
HG = 4            # heads per core
HD = HG * Dh      # 256 output channels per core
SCALE = float(1.0 / np.sqrt(np.float32(1024.0)))
WS = 32.0         # host scale on W_q/W_k so fp8 sees ~unit-variance weights
SCALE_QK = SCALE / (WS * WS)
ROPE_BASE = 10000.0
NCHUNK = S // 512     # 4 s-chunks of 512
NTB = S // 128        # 16 t-blocks of 128
SHUF16 = list(range(16, 32)) + list(range(0, 16))

Exp = mybir.ActivationFunctionType.Exp
Ln = mybir.ActivationFunctionType.Ln
MUL = mybir.AluOpType.mult
ADD = mybir.AluOpType.add
DR = mybir.MatmulPerfMode.DoubleRow


def _build_program(legalize=True):
    nc = bass.Bass("TRN2", target_bir_lowering=False, debug=False)

    xT = nc.dram_tensor("xT", [128, NCHUNK, 8, 512], DT, kind="ExternalInput")
    xT8 = nc.dram_tensor("xT8", [128, NCHUNK, 8, 512], F8, kind="ExternalInput")
    wq8 = nc.dram_tensor("wq8", [128, 8, HD], F8, kind="ExternalInput")
    wk8 = nc.dram_tensor("wk8", [128, 8, HD], F8, kind="ExternalInput")
    wv = nc.dram_tensor("wv", [128, 8, HD], DT, kind="ExternalInput")
    wo = nc.dram_tensor("wo", [128, 2, E], DT, kind="ExternalInput")
    cosd = nc.dram_tensor("cosd", [128, S], DT, kind="ExternalInput")
    sins = nc.dram_tensor("sins", [128, S], DT, kind="ExternalInput")
    trim = nc.dram_tensor("trim", [128, 128], DT, kind="ExternalInput")
    sel2c = nc.dram_tensor("sel2c", [33, 128], F32R, kind="ExternalInput")
    lnz = nc.dram_tensor("lnz", [33, 512], F32R, kind="ExternalInput")
    y = nc.dram_tensor("y", [S, E], DT, kind="ExternalOutput")

    with tile.TileContext(nc) as tc:
        with (
            tc.tile_pool(name="persist", bufs=1) as pp,
            tc.tile_pool(name="xchunks", bufs=3) as xp,
            tc.tile_pool(name="x8chunks", bufs=3) as xp8,
            tc.tile_pool(name="ropetmp", bufs=2) as rt,
            tc.tile_pool(name="att_es", bufs=6) as ep,
            tc.tile_pool(name="att_row", bufs=2) as rp,
            tc.tile_pool(name="osb", bufs=4) as op_,
            tc.tile_pool(name="ystg", bufs=2) as yp,
            tc.tile_pool(name="ps_big", bufs=3, space="PSUM") as psB,
            tc.tile_pool(name="ps_ot", bufs=2, space="PSUM") as psO,
        ):
            # ---- persistent tensors ----
            # Qr^T zero-padded per head half: qz[:, hi, blk, s] has rows of
            # head 2*blk+hi live and the other 64 rows zero, so the score
            # contraction runs over the full 128 partitions.
            qz = pp.tile([128, 2, 2, S], DT)
            krt = pp.tile([128, 2, S], DT)   # Kr^T
            vau = pp.tile([128, NTB, HG, 65], DT)  # V + ones col per (tb, h)
            ot = pp.tile([128, 2, S], DT)    # O^T normalized

            # stationary for PE-warming matmuls
            hW = pp.tile([128, 128], DT)
            nc.vector.memset(hW[:], 1.0)
            wrm = pp.tile([1, 16], DT)
            # single shared ln-rows tile: its write (pi2) / broadcast-read
            # (pi6) windows alternate without overlap; zero rows once so the
            # selector matmul never reads uninitialized memory
            lnr_sh = pp.tile([33, 512], F32R)

            def heat(n=10):
                # full-array 128x128 matmuls to trip the HAM activity window
                # back to K=8/8. Scratch lands in a big-ring PSUM slot whose
                # next real matmul uses start=True and overwrites it.
                htile = psB.tile([128, 128], F32, tag="big", name="heat")
                for _ in range(n):
                    nc.tensor.matmul(htile[:], hW[:], hW[:],
                                     start=True, stop=True)

            # Initial loads fan out over both hardware DMA queues so the
            # first-needed tensors don't wait behind the rest.
            # ordered by first use: Q-proj inputs, RoPE tables, K, then
            # the attention-phase tensors (V/x bf16, masks, W_o)
            wq8_sb = pp.tile([128, 8, HD], F8)
            nc.sync.dma_start(wq8_sb[:], wq8.ap())
            xc80 = xp8.tile([128, 8, 512], F8, tag="xc8", name="xc80")
            nc.scalar.dma_start(xc80[:], xT8.ap()[:, 0, :, :])
            cos_sb = pp.tile([128, S], DT)
            nc.sync.dma_start(cos_sb[:], cosd.ap())
            sin_sb = pp.tile([128, S], DT)
            nc.scalar.dma_start(sin_sb[:], sins.ap())
            wk8_sb = pp.tile([128, 8, HD], F8)
            nc.sync.dma_start(wk8_sb[:], wk8.ap())
            wv_sb = pp.tile([128, 8, HD], DT)
            nc.scalar.dma_start(wv_sb[:], wv.ap())
            xc0 = xp.tile([128, 8, 512], DT, tag="xc", name="xc0")
            nc.sync.dma_start(xc0[:, 0:4, :], xT.ap()[:, 0, 0:4, :])
            nc.scalar.dma_start(xc0[:, 4:8, :], xT.ap()[:, 0, 4:8, :])
            tri_sb = pp.tile([128, 128], DT)
            nc.scalar.dma_start(tri_sb[:], trim.ap())
            sel2_sb = pp.tile([33, 128], F32R)
            nc.scalar.dma_start(sel2_sb[:], sel2c.ap())
            wo_sb = pp.tile([128, 2, E], DT)
            nc.scalar.dma_start(wo_sb[:], wo.ap())

            nc.scalar.dma_start(lnr_sh[:], lnz.ap())
            nc.gpsimd.memset(qz[64:128, 0, :, :], 0.0)
            nc.gpsimd.memset(qz[0:64, 1, :, :], 0.0)
            nc.gpsimd.memset(vau[:, :, :, 64:65], 1.0)

            # preload the ACT exp/ln tables while DMAs stream (first real
            # activation otherwise eats a 1.3us ACT_TABLE_LOAD); writes to a
            # dedicated scratch so nothing waits on the slow first GpSimd op
            with nc.allow_low_precision(reason="ACT table warm"):
                nc.scalar.activation(wrm[:], hW[0:1, 0:16], Exp, bias=0.0, scale=1.0)
                nc.scalar.activation(wrm[:], hW[0:1, 0:16], Ln, bias=0.0, scale=1.0)

            # warm the PE during the initial DMA streams
            heat(n=58)

            def emit_ln(osb, lnr):
                # ln of the two softmax denominator rows into adjacent
                # partitions of one lnr tile (from the drained osb rows)
                for hi in range(2):
                    nc.scalar.activation(lnr[32 * hi:32 * hi + 1, :],
                                         osb[hi][64:65, :], Ln, bias=0.0, scale=1.0)

            def emit_ln_psum(otp, lnr):
                # tail path: ln straight off the PSUM accumulator rows
                for hi in range(2):
                    nc.scalar.activation(lnr[32 * hi:32 * hi + 1, :],
                                         otp[hi][64:65, :], Ln, bias=0.0, scale=1.0)

            def emit_bcmult(sc, hp, osb, lnr):
                # broadcast the two ln rows with one selector matmul, one
                # full-width exp(-x) recovers the reciprocals, then DVE
                # scales the O rows into ot. Emitted several blocks after
                # emit_ln so the bc matmul never head-of-line-blocks the
                # in-order PE queue.
                ss = slice(sc * 512, (sc + 1) * 512)
                bc = psB.tile([128, 512], F32, tag="big", name="bc")
                nc.tensor.matmul(bc[:], sel2_sb[:], lnr[:], start=True, stop=True)
                ew = rp.tile([64, 2, 512], DT, tag="ew", name="ew")
                with nc.allow_low_precision(reason="recip broadcast"):
                    for hi in range(2):
                        nc.scalar.activation(ew[:, hi, :],
                                             bc[hi * 64:(hi + 1) * 64, :],
                                             Exp, bias=0.0, scale=-1.0)
                for hi in range(2):
                    with nc.allow_low_precision(reason="normalized O rows"):
                        nc.vector.tensor_tensor(ot[hi * 64:(hi + 1) * 64, hp, ss],
                                                osb[hi][0:64, :],
                                                ew[:, hi, :], MUL)

            def emit_wo_sbl(sc, sbl, tail_heat=False):
                # W_o partials for one 128-row s-block of chunk sc; both
                # 512-col E halves accumulate in one [128,1024] PSUM pair.
                sb_i = sc * 4 + sbl
                tsl = slice(sb_i * 128, (sb_i + 1) * 128)
                py = psB.tile([128, 1024], F32, tag="big", name="py")
                for ec in range(2):
                    for blk in range(2):
                        nc.tensor.matmul(
                            py[:, ec * 512:(ec + 1) * 512], ot[:, blk, tsl],
                            wo_sb[:, blk, ec * 512:(ec + 1) * 512],
                            start=(blk == 0), stop=(blk == 1),
                        )
                ystg = yp.tile([128, E], DT, tag="y")
                with nc.allow_low_precision(reason="partial sum staging"):
                    if tail_heat:
                        # tail: DVE runs the normalize mults; stage on ACT
                        nc.scalar.copy(ystg[:], py[:])
                        heat(n=2)
                    else:
                        nc.vector.tensor_copy(ystg[:], py[:])
                nc.sync.dma_start(y.ap()[tsl, :], ystg[:])

            def emit_wo(sc):
                for sbl in range(4):
                    emit_wo_sbl(sc, sbl, tail_heat=(sc == 3))

            pend_norm = None  # (sc, osb) for hp=1, normalized next chunk
            osb_prev = {}

            def dma_chunk(sc):
                xc = xp.tile([128, 8, 512], DT, tag="xc", name=f"xc{sc}")
                nc.sync.dma_start(xc[:], xT.ap()[:, sc, :, :])
                xc8 = xp8.tile([128, 8, 512], F8, tag="xc8", name=f"xc8{sc}")
                nc.scalar.dma_start(xc8[:], xT8.ap()[:, sc, :, :])
                return xc, xc8

            def vproj(sc, xc, tbl):
                # V rows for one of chunk sc's 4 new t-blocks; only the
                # diagonal PVs at the end of this chunk's tb loop need them.
                tb = sc * 4 + tbl
                pv = psB.tile([128, 256], F32, tag="big", name="pv")
                for e in range(8):
                    nc.tensor.matmul(
                        pv[:], xc[:, e, tbl * 128:(tbl + 1) * 128],
                        wv_sb[:, e, :], start=(e == 0), stop=(e == 7),
                    )
                with nc.allow_low_precision(reason="rounded matmul input"):
                    nc.vector.tensor_copy(
                        vau[:, tb, :, 0:64],
                        pv[:].rearrange("p (h d) -> p h d", d=64),
                    )

            def qkproj_mb(sc, xc8, w_sb, dst, mb):
                # one 128-chan half of a fp8 DoubleRow projection + its RoPE
                # for chunk sc; dst=None -> qz (zero-padded halves), else
                # krt. mb0 feeds blk0 (hp0) scores, mb1 feeds hp1 -- so mb1
                # pieces can be emitted much later.
                ss = slice(sc * 512, (sc + 1) * 512)
                pq = psB.tile([128, 512], F32, tag="big", name="pq")
                for ktp in range(4):
                    nc.tensor.matmul(
                        pq[:],
                        w_sb[:, 2 * ktp:2 * ktp + 2, mb * 128:(mb + 1) * 128],
                        xc8[:, 2 * ktp:2 * ktp + 2, :],
                        start=(ktp == 0), stop=(ktp == 3),
                        perf_mode=DR,
                    )
                a = rt.tile([128, 512], DT, tag="a")
                with nc.allow_low_precision(reason="rounded matmul input"):
                    nc.vector.tensor_copy(a[:], pq[:])
                bsh = rt.tile([128, 512], DT, tag="b")
                nc.vector.stream_shuffle(bsh[:], a[:], SHUF16)
                t1 = rt.tile([128, 512], DT, tag="t1")
                t2 = rt.tile([128, 512], DT, tag="t2")
                with nc.allow_low_precision(reason="rounded matmul input"):
                    nc.vector.tensor_tensor(t1[:], bsh[:], sin_sb[:, ss], MUL)
                    nc.vector.tensor_tensor(t2[:], a[:], cos_sb[:, ss], MUL)
                    if dst is None:  # Q: split into zero-padded halves
                        nc.vector.tensor_tensor(
                            qz[0:64, 0, mb, ss], t2[0:64, :], t1[0:64, :], ADD)
                        nc.vector.tensor_tensor(
                            qz[64:128, 1, mb, ss], t2[64:128, :], t1[64:128, :], ADD)
                    else:
                        nc.vector.tensor_tensor(
                            dst[:, mb, ss], t2[:, :], t1[:, :], ADD)

            # ---- chunk 0 prologue: mb0 halves first so hp0 scores can
            # start while the mb1 halves are still in flight.
            xcs = {0: (xc0, xc80)}
            qkproj_mb(0, xc80, wq8_sb, None, 0)
            qkproj_mb(0, xc80, wk8_sb, krt, 0)
            # chunk 0's V projection runs here, filling the PE while the DVE
            # RoPE chains for the mb0 halves drain; its vau tiles are only
            # needed once attention starts.
            for tbl in range(4):
                vproj(0, xc0, tbl)
            qkproj_mb(0, xc80, wq8_sb, None, 1)
            qkproj_mb(0, xc80, wk8_sb, krt, 1)


            for sc in range(NCHUNK):
                ntb = 4 * sc + 4
                xc, xc8 = xcs[sc]

                # ---- attention for this chunk, one head-pair at a time,
                # with next-chunk projection, V, W_o and normalize chains
                # woven between attention blocks so no engine sees a burst.
                osb_h = [None, None]
                fulls = list(range(4 * sc))
                diags = [4 * sc + i for i in range(4)]
                if sc == 0:
                    order = diags
                else:
                    k = len(fulls) // 4
                    order = []
                    fi = 0
                    for i, d in enumerate(diags):
                        take = max(2, k) if i == 0 else k
                        take = min(take, len(fulls) - fi)
                        order += fulls[fi:fi + take]
                        fi += take
                        order.append(d)
                    order += fulls[fi:]
                qk_step = max(ntb // 4, 1)

                for hp in range(2):
                    otp = [psO.tile([65, 512], F32, tag="ot", name=f"otp{hi}")
                           for hi in range(2)]
                    blk = hp
                    wo_step = max(ntb // 4, 2)
                    pv_prev = None  # (pi, tb, lo, es)
                    for pi, tb in enumerate(order):
                        m = tb - 4 * sc
                        lo = 128 * max(m, 0)  # diag: skip cols left of block
                        if hp == 0:
                            if pi == 0 and sc < 3:
                                xcs[sc + 1] = dma_chunk(sc + 1)
                            if pi < 4 and sc > 0:
                                vproj(sc, xc, pi)
                            if pi == 1 and sc == 1:
                                # chunk 0 hp0's bc didn't fit in its 4-block
                                # hp1 loop; runs here
                                emit_bcmult(0, 0, *osb_prev[(0, 0)])
                            if pi == 2 and pend_norm is not None:
                                emit_ln(pend_norm[1][0], pend_norm[1][1])
                            if pi == 6 and pend_norm is not None:
                                emit_bcmult(pend_norm[0], 1, *pend_norm[1])
                                pend_norm = None
                        else:
                            if pi == 2:
                                # deferred normalize of this chunk's hp=0
                                emit_ln(osb_h[0][0], osb_h[0][1])
                            if pi == 6:
                                emit_bcmult(sc, 0, *osb_h[0])
                            if (sc > 0 and pi >= 1
                                    and (pi - 1) % wo_step == 0
                                    and (pi - 1) // wo_step < 4):
                                # W_o of the previous chunk, one s-block at
                                # a time
                                emit_wo_sbl(sc - 1, (pi - 1) // wo_step)
                            if sc < 3 and pi % qk_step == 0 and pi // qk_step < 4:
                                # next chunk's projection, one (proj, mb)
                                # piece at a time: Q-mb0, K-mb0, Q-mb1, K-mb1
                                j = pi // qk_step
                                w_sb_n = (wq8_sb, wk8_sb)[j % 2]
                                dst_n = (None, krt)[j % 2]
                                qkproj_mb(sc + 1, xcs[sc + 1][1], w_sb_n,
                                          dst_n, j // 2)
                        pss = psB.tile([128, 1024], F32, tag="big", name="pss")
                        ps3 = pss[:].rearrange("p (h s) -> p h s", h=2)
                        for hi in range(2):
                            nc.tensor.matmul(
                                ps3[:, hi, lo:512],
                                krt[:, blk, tb * 128:(tb + 1) * 128],
                                qz[:, hi, blk, sc * 512 + lo:(sc + 1) * 512],
                                start=True, stop=True,
                            )
                        es = ep.tile([128, 1024], DT, tag="es", name="es")
                        es3 = es[:].rearrange("p (h s) -> p h s", h=2)
                        with nc.allow_low_precision(reason="rounded matmul input"):
                            nc.scalar.activation(es3[:, :, lo:512], ps3[:, :, lo:512],
                                                 Exp, bias=0.0, scale=SCALE_QK)
                        if m >= 0:  # mask the diagonal 128-col triangle
                            trib = tri_sb[:].rearrange("p (o s) -> p o s", o=1).to_broadcast((128, 2, 128))
                            with nc.allow_low_precision(reason="rounded matmul input"):
                                nc.vector.tensor_tensor(
                                    es3[:, :, lo:lo + 128], es3[:, :, lo:lo + 128],
                                    trib, MUL)
                        # PV trails the score stream by one block: the PE
                        # always has the next scores queued ahead of a PV
                        # that may still be waiting on its exp.
                        if pv_prev is not None:
                            ppi, ptb, plo, pes = pv_prev
                            for hi in range(2):
                                nc.tensor.matmul(
                                    otp[hi][:, plo:512],
                                    vau[:, ptb, 2 * hp + hi, :],
                                    pes[:, hi * 512 + plo:(hi + 1) * 512],
                                    start=(ppi == 0), stop=False,
                                    skip_group_check=True,
                                )
                        pv_prev = (pi, tb, lo, es)
                    ppi, ptb, plo, pes = pv_prev
                    for hi in range(2):
                        nc.tensor.matmul(
                            otp[hi][:, plo:512], vau[:, ptb, 2 * hp + hi, :],
                            pes[:, hi * 512 + plo:(hi + 1) * 512],
                            start=(ppi == 0), stop=True,
                            skip_group_check=True,
                        )
                    if sc == 3 and hp == 1:
                        # tail: start the ln chain straight off PSUM so it
                        # overlaps the accumulator drain
                        emit_ln_psum(otp, lnr_sh)
                    # drain O accumulators to SBUF promptly so the two PSUM
                    # banks recycle for the next head-pair; the two copies
                    # run on different engines so the drain latency halves.
                    osb = [op_.tile([65, 512], DT, tag="osb", name=f"osb{hi}")
                           for hi in range(2)]
                    with nc.allow_low_precision(reason="pre-normalize O"):
                        nc.vector.tensor_copy(osb[0][:], otp[0][:])
                        nc.scalar.copy(osb[1][:], otp[1][:])
                    osb_h[hp] = (osb, lnr_sh)

                osb_prev[(sc, 0)] = osb_h[0]
                if sc == 3:
                    # ln already emitted off PSUM at the drain above; heats
                    # keep the PE fed while the ACT chain completes.
                    heat(n=20)
                    emit_bcmult(3, 1, osb_h[1][0], lnr_sh)
                    emit_wo(3)
                else:
                    pend_norm = (sc, osb_h[1])

    if legalize:
        _legalize_waits(nc)
    return nc


def _legalize_waits(nc, max_waits=1):
    """Split >max_waits sync waits onto preceding same-engine NoOps
    (several instruction encodings only have one sync-wait slot)."""
    for fn in nc.m.functions:
        for bb in fn.blocks:
            new_insts = []
            for inst in bb.instructions:
                si = inst.sync_info
                waits = list(si.on_wait) if si is not None and si.on_wait else []
                if len(waits) > max_waits:
                    carry, keep = waits[:-max_waits], waits[-max_waits:]
                    for i, w in enumerate(carry):
                        new_insts.append(mybir.InstNoOp(
                            name=f"{inst.name}_wsplit{i}",
                            engine=inst.engine,
                            bass_nofuse=True,
                            sync_info=mybir.SyncInfo(on_wait=[w], on_update=[]),
                        ))
                    si.on_wait = keep
                new_insts.append(inst)
            bb.instructions[:] = new_insts


def _host_constants():
    # RoPE channel permutation: row r (within a head, 0..63) holds source
    # channel d = 2*i + odd with i = 16*(r//32) + r%16, odd = (r%32)//16.
    r = np.arange(64)
    i_ = 16 * (r // 32) + (r % 16)
    odd = (r % 32) // 16
    dsrc = 2 * i_ + odd  # source channel per permuted row

    inv_freq = ROPE_BASE ** (-(i_.astype(np.float64)) * 2.0 / Dh)
    ang = np.arange(S, dtype=np.float64)[None, :] * inv_freq[:, None]  # [64, S]
    cos64 = np.cos(ang)
    sin64 = np.sin(ang) * np.where(odd == 0, -1.0, 1.0)[:, None]
    cosd = np.tile(cos64, (2, 1)).astype(DT_NP)
    sins = np.tile(sin64, (2, 1)).astype(DT_NP)

    t = np.arange(128)[:, None]
    s = np.arange(128)[None, :]
    trim = (t <= s).astype(DT_NP)

    sel2 = np.zeros((33, 128), np.float32)
    sel2[0, 0:64] = 1
    sel2[32, 64:128] = 1
    return dsrc, cosd, sins, trim, sel2


def _wlay(w, dt=None):  # [E, HD] -> [p, ko, m] contiguous
    return np.ascontiguousarray(w.reshape(8, 128, HD).transpose(1, 0, 2)).astype(dt or DT_NP)


def _wolay(w):  # [HD, E] -> [p, ko, e] contiguous
    return np.ascontiguousarray(w.reshape(2, 128, E).transpose(1, 0, 2)).astype(DT_NP)


_CACHE = {}


def _run(inputs, trace=False):
    if "nc" not in _CACHE:
        _CACHE["nc"] = _build_program()
        _CACHE["consts"] = _host_constants()
    nc = _CACHE["nc"]
    dsrc, cosd, sins, trim, sel2 = _CACHE["consts"]

    x = np.ascontiguousarray(np.asarray(inputs["x"]), dtype=np.float32)
    W_q = np.asarray(inputs["W_q"], dtype=np.float32)
    W_k = np.asarray(inputs["W_k"], dtype=np.float32)
    W_v = np.asarray(inputs["W_v"], dtype=np.float32)
    W_o = np.asarray(inputs["W_o"], dtype=np.float32)

    # [p, sc, eo, s] so each chunk DMA is contiguous per partition
    xTc = [np.ascontiguousarray(
        x[b].reshape(NCHUNK, 512, 8, 128).transpose(3, 0, 2, 1))
        for b in range(B)]
    xTb = [t.astype(DT_NP) for t in xTc]
    xT8 = [t.astype(F8_NP) for t in xTc]

    in_maps = []
    for c in range(8):
        b, g = divmod(c, 4)
        heads = np.arange(4 * g, 4 * g + 4)
        rows_qk = (heads[:, None] * 64 + dsrc[None, :]).reshape(-1)   # permuted
        rows_v = (heads[:, None] * 64 + np.arange(64)[None, :]).reshape(-1)
        in_maps.append({
            "xT": xTb[b],
            "xT8": xT8[b],
            "wq8": _wlay(W_q[rows_qk].T * WS, F8_NP),
            "wk8": _wlay(W_k[rows_qk].T * WS, F8_NP),
            "wv": _wlay(W_v[rows_v].T),
            "wo": _wolay(W_o[:, rows_v].T),
            "cosd": cosd, "sins": sins, "trim": trim, "sel2c": sel2,
            "lnz": np.zeros((33, 512), np.float32),
        })

    res = bass_utils.run_bass_kernel_spmd(
        nc, in_maps, core_ids=list(range(8)), trace=trace,
    )
    out = np.zeros((B, S, E), np.float32)
    for c in range(8):
        out[c // 4] += res.results[c]["y"].astype(np.float32)
    return out, res


def kernel(**inputs):
    out, _ = _run(inputs, trace=False)
    return out


# revision 60
# speedup vs baseline: 1.0065x; 1.0022x over previous
"""Multi-head causal attention with RoPE on 8 TRN2 NeuronCores.

Sharding: batch (2) x head-groups (4 of 4 heads) -> 8 cores; host sums the
4 per-batch partial y's.

Per core, a software-pipelined stream over 512-row s-chunks:
- Q/K projections are fp8-e4m3 DoubleRow matmuls (2x PE throughput over the
  K=1024 contraction; host pre-casts x and 32-scaled W_q/W_k to fp8, the
  1024x score growth folds into the softmax exp scale). V projection,
  scores, PV and W_o stay bf16: fp8 on any of those overflows the 2e-2
  error gate, and scores/PV are PSUM-write-port bound at Dh=64 anyway.
- Transposed scores S^T = Kr @ Qr^T with both heads of a pair in one
  [128,1024] PSUM tile; causal 128-block skip with column-subrange matmuls
  and a post-exp triangle mask on diagonal blocks; PV with a ones-column on
  V accumulates the softmax denominator; PV trails the score stream by one
  block so the in-order PE never waits on an exp.
- Everything non-attention is woven between attention blocks, never in
  bursts: next-chunk x DMA + V projection into hp0, next-chunk Q/K
  projection+RoPE (quarter pieces: the mb0 half feeds hp0 scores, mb1 can
  land late) and prev-chunk W_o (one s-block at a time) into hp1, and the
  diagonal t-blocks are interleaved among full blocks so the cheap ones
  never bunch up.
- The softmax normalize is split into stages emitted far apart so its
  cross-engine chain never head-of-line-blocks an engine queue: ACT ln of
  the two denominator rows into a shared f32r tile, a selector matmul
  broadcasting them 2 blocks later, one wide exp(-x), then DVE scales into
  ot. W_o consumes ot a half-chunk behind.
- bf16 throughout with f32 PSUM accumulation; fp8 only where noted.
"""
import os
import sys

sys.path.insert(0, "/opt/trn_rl_repo")

import ml_dtypes
import numpy as np

import concourse.bass as bass
import concourse.mybir as mybir
import concourse.tile as tile
from concourse import bass_utils

F32 = mybir.dt.float32
BF16 = mybir.dt.bfloat16
F8 = mybir.dt.float8e4
F32R = mybir.dt.float32r

DT = BF16
DT_NP = ml_dtypes.bfloat16
F8_NP = ml_dtypes.float8_e4m3fn

B, S, E, H, Dh = 2, 2048, 1024, 16, 64
HG = 4            # heads per core
HD = HG * Dh      # 256 output channels per core
SCALE = float(1.0 / np.sqrt(np.float32(1024.0)))
WS = 32.0         # host scale on W_q/W_k so fp8 sees ~unit-variance weights
SCALE_QK = SCALE / (WS * WS)
ROPE_BASE = 10000.0
NCHUNK = S // 512     # 4 s-chunks of 512
NTB = S // 128        # 16 t-blocks of 128
SHUF16 = list(range(16, 32)) + list(range(0, 16))

Exp = mybir.ActivationFunctionType.Exp
Ln = mybir.ActivationFunctionType.Ln
MUL = mybir.AluOpType.mult
ADD = mybir.AluOpType.add
DR = mybir.MatmulPerfMode.DoubleRow


def _build_program(legalize=True):
    nc = bass.Bass("TRN2", target_bir_lowering=False, debug=False)

    xT = nc.dram_tensor("xT", [128, NCHUNK, 8, 512], DT, kind="ExternalInput")
    xT8 = nc.dram_tensor("xT8", [128, NCHUNK, 8, 512], F8, kind="ExternalInput")
    wq8 = nc.dram_tensor("wq8", [128, 8, HD], F8, kind="ExternalInput")
    wk8 = nc.dram_tensor("wk8", [128, 8, HD], F8, kind="ExternalInput")
    wv = nc.dram_tensor("wv", [128, 8, HD], DT, kind="ExternalInput")
    wo = nc.dram_tensor("wo", [128, 2, E], DT, kind="ExternalInput")
    cosd = nc.dram_tensor("cosd", [128, S], DT, kind="ExternalInput")
    sins = nc.dram_tensor("sins", [128, S], DT, kind="ExternalInput")
    trim = nc.dram_tensor("trim", [128, 128], DT, kind="ExternalInput")
    sel2c = nc.dram_tensor("sel2c", [33, 128], F32R, kind="ExternalInput")
    lnz = nc.dram_tensor("lnz", [33, 512], F32R, kind="ExternalInput")
    y = nc.dram_tensor("y", [S, E], DT, kind="ExternalOutput")

    with tile.TileContext(nc) as tc:
        with (
            tc.tile_pool(name="persist", bufs=1) as pp,
            tc.tile_pool(name="xchunks", bufs=3) as xp,
            tc.tile_pool(name="x8chunks", bufs=3) as xp8,
            tc.tile_pool(name="ropetmp", bufs=2) as rt,
            tc.tile_pool(name="att_es", bufs=6) as ep,
            tc.tile_pool(name="att_row", bufs=2) as rp,
            tc.tile_pool(name="osb", bufs=4) as op_,
            tc.tile_pool(name="ystg", bufs=2) as yp,
            tc.tile_pool(name="ps_big", bufs=3, space="PSUM") as psB,
            tc.tile_pool(name="ps_ot", bufs=2, space="PSUM") as psO,
        ):
            # ---- persistent tensors ----
            # Qr^T zero-padded per head half: qz[:, hi, blk, s] has rows of
            # head 2*blk+hi live and the other 64 rows zero, so the score
            # contraction runs over the full 128 partitions.
            qz = pp.tile([128, 2, 2, S], DT)
            krt = pp.tile([128, 2, S], DT)   # Kr^T
            vau = pp.tile([128, NTB, HG, 65], DT)  # V + ones col per (tb, h)
            ot = pp.tile([128, 2, S], DT)    # O^T normalized

            # stationary for PE-warming matmuls
            hW = pp.tile([128, 128], DT)
            nc.vector.memset(hW[:], 1.0)
            wrm = pp.tile([1, 16], DT)
            # single shared ln-rows tile: its write (pi2) / broadcast-read
            # (pi6) windows alternate without overlap; zero rows once so the
            # selector matmul never reads uninitialized memory
            lnr_sh = pp.tile([33, 512], F32R)

            def heat(n=10):
                # full-array 128x128 matmuls to trip the HAM activity window
                # back to K=8/8. Scratch lands in a big-ring PSUM slot whose
                # next real matmul uses start=True and overwrites it.
                htile = psB.tile([128, 128], F32, tag="big", name="heat")
                for _ in range(n):
                    nc.tensor.matmul(htile[:], hW[:], hW[:],
                                     start=True, stop=True)

            # Initial loads fan out over both hardware DMA queues so the
            # first-needed tensors don't wait behind the rest.
            # ordered by first use: Q-proj inputs, RoPE tables, K, then
            # the attention-phase tensors (V/x bf16, masks, W_o)
            wq8_sb = pp.tile([128, 8, HD], F8)
            nc.sync.dma_start(wq8_sb[:], wq8.ap())
            xc80 = xp8.tile([128, 8, 512], F8, tag="xc8", name="xc80")
            nc.scalar.dma_start(xc80[:], xT8.ap()[:, 0, :, :])
            cos_sb = pp.tile([128, S], DT)
            nc.sync.dma_start(cos_sb[:], cosd.ap())
            sin_sb = pp.tile([128, S], DT)
            nc.scalar.dma_start(sin_sb[:], sins.ap())
            wk8_sb = pp.tile([128, 8, HD], F8)
            nc.sync.dma_start(wk8_sb[:], wk8.ap())
            wv_sb = pp.tile([128, 8, HD], DT)
            nc.scalar.dma_start(wv_sb[:], wv.ap())
            xc0 = xp.tile([128, 8, 512], DT, tag="xc", name="xc0")
            nc.sync.dma_start(xc0[:, 0:4, :], xT.ap()[:, 0, 0:4, :])
            nc.scalar.dma_start(xc0[:, 4:8, :], xT.ap()[:, 0, 4:8, :])
            tri_sb = pp.tile([128, 128], DT)
            nc.scalar.dma_start(tri_sb[:], trim.ap())
            sel2_sb = pp.tile([33, 128], F32R)
            nc.scalar.dma_start(sel2_sb[:], sel2c.ap())
            wo_sb = pp.tile([128, 2, E], DT)
            nc.scalar.dma_start(wo_sb[:], wo.ap())

            nc.scalar.dma_start(lnr_sh[:], lnz.ap())
            nc.gpsimd.memset(qz[64:128, 0, :, :], 0.0)
            nc.gpsimd.memset(qz[0:64, 1, :, :], 0.0)
            nc.gpsimd.memset(vau[:, :, :, 64:65], 1.0)

            # preload the ACT exp/ln tables while DMAs stream (first real
            # activation otherwise eats a 1.3us ACT_TABLE_LOAD); writes to a
            # dedicated scratch so nothing waits on the slow first GpSimd op
            with nc.allow_low_precision(reason="ACT table warm"):
                nc.scalar.activation(wrm[:], hW[0:1, 0:16], Exp, bias=0.0, scale=1.0)
                nc.scalar.activation(wrm[:], hW[0:1, 0:16], Ln, bias=0.0, scale=1.0)

            # warm the PE during the initial DMA streams
            heat(n=58)

            def emit_ln(osb, lnr):
                # ln of the two softmax denominator rows into adjacent
                # partitions of one lnr tile (from the drained osb rows)
                for hi in range(2):
                    nc.scalar.activation(lnr[32 * hi:32 * hi + 1, :],
                                         osb[hi][64:65, :], Ln, bias=0.0, scale=1.0)

            def emit_ln_psum(otp, lnr):
                # tail path: ln straight off the PSUM accumulator rows
                for hi in range(2):
                    nc.scalar.activation(lnr[32 * hi:32 * hi + 1, :],
                                         otp[hi][64:65, :], Ln, bias=0.0, scale=1.0)

            def emit_bcmult(sc, hp, osb, lnr):
                # broadcast the two ln rows with one selector matmul, one
                # full-width exp(-x) recovers the reciprocals, then DVE
                # scales the O rows into ot. Emitted several blocks after
                # emit_ln so the bc matmul never head-of-line-blocks the
                # in-order PE queue.
                ss = slice(sc * 512, (sc + 1) * 512)
                bc = psB.tile([128, 512], F32, tag="big", name="bc")
                nc.tensor.matmul(bc[:], sel2_sb[:], lnr[:], start=True, stop=True)
                ew = rp.tile([64, 2, 512], DT, tag="ew", name="ew")
                with nc.allow_low_precision(reason="recip broadcast"):
                    for hi in range(2):
                        nc.scalar.activation(ew[:, hi, :],
                                             bc[hi * 64:(hi + 1) * 64, :],
                                             Exp, bias=0.0, scale=-1.0)
                for hi in range(2):
                    with nc.allow_low_precision(reason="normalized O rows"):
                        nc.vector.tensor_tensor(ot[hi * 64:(hi + 1) * 64, hp, ss],
                                                osb[hi][0:64, :],
                                                ew[:, hi, :], MUL)

            def emit_wo_sbl(sc, sbl, tail_heat=False):
                # W_o partials for one 128-row s-block of chunk sc; both
                # 512-col E halves accumulate in one [128,1024] PSUM pair.
                sb_i = sc * 4 + sbl
                tsl = slice(sb_i * 128, (sb_i + 1) * 128)
                py = psB.tile([128, 1024], F32, tag="big", name="py")
                for ec in range(2):
                    for blk in range(2):
                        nc.tensor.matmul(
                            py[:, ec * 512:(ec + 1) * 512], ot[:, blk, tsl],
                            wo_sb[:, blk, ec * 512:(ec + 1) * 512],
                            start=(blk == 0), stop=(blk == 1),
                        )
                ystg = yp.tile([128, E], DT, tag="y")
                with nc.allow_low_precision(reason="partial sum staging"):
                    if tail_heat:
                        # tail: DVE runs the normalize mults; stage on ACT
                        nc.scalar.copy(ystg[:], py[:])
                        heat(n=2)
                    else:
                        nc.vector.tensor_copy(ystg[:], py[:])
                nc.sync.dma_start(y.ap()[tsl, :], ystg[:])

            def emit_wo(sc):
                for sbl in range(4):
                    emit_wo_sbl(sc, sbl, tail_heat=(sc == 3))

            pend_norm = None  # (sc, osb) for hp=1, normalized next chunk
            osb_prev = {}

            def dma_chunk(sc):
                xc = xp.tile([128, 8, 512], DT, tag="xc", name=f"xc{sc}")
                nc.sync.dma_start(xc[:], xT.ap()[:, sc, :, :])
                xc8 = xp8.tile([128, 8, 512], F8, tag="xc8", name=f"xc8{sc}")
                nc.scalar.dma_start(xc8[:], xT8.ap()[:, sc, :, :])
                return xc, xc8

            def vproj(sc, xc, tbl):
                # V rows for one of chunk sc's 4 new t-blocks; only the
                # diagonal PVs at the end of this chunk's tb loop need them.
                tb = sc * 4 + tbl
                pv = psB.tile([128, 256], F32, tag="big", name="pv")
                for e in range(8):
                    nc.tensor.matmul(
                        pv[:], xc[:, e, tbl * 128:(tbl + 1) * 128],
                        wv_sb[:, e, :], start=(e == 0), stop=(e == 7),
                    )
                with nc.allow_low_precision(reason="rounded matmul input"):
                    nc.vector.tensor_copy(
                        vau[:, tb, :, 0:64],
                        pv[:].rearrange("p (h d) -> p h d", d=64),
                    )

            def qkproj_mb(sc, xc8, w_sb, dst, mb):
                # one 128-chan half of a fp8 DoubleRow projection + its RoPE
                # for chunk sc; dst=None -> qz (zero-padded halves), else
                # krt. mb0 feeds blk0 (hp0) scores, mb1 feeds hp1 -- so mb1
                # pieces can be emitted much later.
                ss = slice(sc * 512, (sc + 1) * 512)
                pq = psB.tile([128, 512], F32, tag="big", name="pq")
                for ktp in range(4):
                    nc.tensor.matmul(
                        pq[:],
                        w_sb[:, 2 * ktp:2 * ktp + 2, mb * 128:(mb + 1) * 128],
                        xc8[:, 2 * ktp:2 * ktp + 2, :],
                        start=(ktp == 0), stop=(ktp == 3),
                        perf_mode=DR,
                    )
                a = rt.tile([128, 512], DT, tag="a")
                with nc.allow_low_precision(reason="rounded matmul input"):
                    nc.vector.tensor_copy(a[:], pq[:])
                bsh = rt.tile([128, 512], DT, tag="b")
                nc.vector.stream_shuffle(bsh[:], a[:], SHUF16)
                t1 = rt.tile([128, 512], DT, tag="t1")
                t2 = rt.tile([128, 512], DT, tag="t2")
                with nc.allow_low_precision(reason="rounded matmul input"):
                    nc.vector.tensor_tensor(t1[:], bsh[:], sin_sb[:, ss], MUL)
                    nc.vector.tensor_tensor(t2[:], a[:], cos_sb[:, ss], MUL)
                    if dst is None:  # Q: split into zero-padded halves
                        nc.vector.tensor_tensor(
                            qz[0:64, 0, mb, ss], t2[0:64, :], t1[0:64, :], ADD)
                        nc.vector.tensor_tensor(
                            qz[64:128, 1, mb, ss], t2[64:128, :], t1[64:128, :], ADD)
                    else:
                        nc.vector.tensor_tensor(
                            dst[:, mb, ss], t2[:, :], t1[:, :], ADD)

            # ---- chunk 0 prologue: mb0 halves first so hp0 scores can
            # start while the mb1 halves are still in flight.
            xcs = {0: (xc0, xc80)}
            qkproj_mb(0, xc80, wq8_sb, None, 0)
            qkproj_mb(0, xc80, wk8_sb, krt, 0)
            # chunk 0's V projection runs here, filling the PE while the DVE
            # RoPE chains for the mb0 halves drain; its vau tiles are only
            # needed once attention starts.
            for tbl in range(4):
                vproj(0, xc0, tbl)
            qkproj_mb(0, xc80, wq8_sb, None, 1)
            qkproj_mb(0, xc80, wk8_sb, krt, 1)


            for sc in range(NCHUNK):
                ntb = 4 * sc + 4
                xc, xc8 = xcs[sc]

                # ---- attention for this chunk, one head-pair at a time,
                # with next-chunk projection, V, W_o and normalize chains
                # woven between attention blocks so no engine sees a burst.
                osb_h = [None, None]
                fulls = list(range(4 * sc))
                diags = [4 * sc + i for i in range(4)]
                if sc == 0:
                    order = diags
                else:
                    k = len(fulls) // 4
                    order = []
                    fi = 0
                    for i, d in enumerate(diags):
                        take = max(2, k) if i == 0 else k
                        take = min(take, len(fulls) - fi)
                        order += fulls[fi:fi + take]
                        fi += take
                        order.append(d)
                    order += fulls[fi:]
                qk_step = max(ntb // 4, 1)

                for hp in range(2):
                    otp = [psO.tile([65, 512], F32, tag="ot", name=f"otp{hi}")
                           for hi in range(2)]
                    blk = hp
                    wo_step = max(ntb // 4, 2)
                    pv_prev = None  # (pi, tb, lo, es)
                    for pi, tb in enumerate(order):
                        m = tb - 4 * sc
                        lo = 128 * max(m, 0)  # diag: skip cols left of block
                        if hp == 0:
                            if pi == 0 and sc < 3:
                                xcs[sc + 1] = dma_chunk(sc + 1)
                            if pi < 4 and sc > 0:
                                vproj(sc, xc, pi)
                            if pi == 1 and sc == 1:
                                # chunk 0 hp0's bc didn't fit in its 4-block
                                # hp1 loop; runs here
                                emit_bcmult(0, 0, *osb_prev[(0, 0)])
                            if pi == 2 and pend_norm is not None:
                                emit_ln(pend_norm[1][0], pend_norm[1][1])
                            if pi == 6 and pend_norm is not None:
                                emit_bcmult(pend_norm[0], 1, *pend_norm[1])
                                pend_norm = None
                        else:
                            if pi == 2:
                                # deferred normalize of this chunk's hp=0
                                emit_ln(osb_h[0][0], osb_h[0][1])
                            if pi == 6:
                                emit_bcmult(sc, 0, *osb_h[0])
                            if (sc > 0 and pi >= 1
                                    and (pi - 1) % wo_step == 0
                                    and (pi - 1) // wo_step < 4):
                                # W_o of the previous chunk, one s-block at
                                # a time
                                emit_wo_sbl(sc - 1, (pi - 1) // wo_step)
                            if sc < 3 and pi % qk_step == 0 and pi // qk_step < 4:
                                # next chunk's projection, one (proj, mb)
                                # piece at a time: Q-mb0, K-mb0, Q-mb1, K-mb1
                                j = pi // qk_step
                                w_sb_n = (wq8_sb, wk8_sb)[j % 2]
                                dst_n = (None, krt)[j % 2]
                                qkproj_mb(sc + 1, xcs[sc + 1][1], w_sb_n,
                                          dst_n, j // 2)
                        pss = psB.tile([128, 1024], F32, tag="big", name="pss")
                        ps3 = pss[:].rearrange("p (h s) -> p h s", h=2)
                        for hi in range(2):
                            nc.tensor.matmul(
                                ps3[:, hi, lo:512],
                                krt[:, blk, tb * 128:(tb + 1) * 128],
                                qz[:, hi, blk, sc * 512 + lo:(sc + 1) * 512],
                                start=True, stop=True,
                            )
                        es = ep.tile([128, 1024], DT, tag="es", name="es")
                        es3 = es[:].rearrange("p (h s) -> p h s", h=2)
                        with nc.allow_low_precision(reason="rounded matmul input"):
                            nc.scalar.activation(es3[:, :, lo:512], ps3[:, :, lo:512],
                                                 Exp, bias=0.0, scale=SCALE_QK)
                        if m >= 0:  # mask the diagonal 128-col triangle
                            trib = tri_sb[:].rearrange("p (o s) -> p o s", o=1).to_broadcast((128, 2, 128))
                            with nc.allow_low_precision(reason="rounded matmul input"):
                                nc.vector.tensor_tensor(
                                    es3[:, :, lo:lo + 128], es3[:, :, lo:lo + 128],
                                    trib, MUL)
                        # PV trails the score stream by one block: the PE
                        # always has the next scores queued ahead of a PV
                        # that may still be waiting on its exp.
                        if pv_prev is not None:
                            ppi, ptb, plo, pes = pv_prev
                            for hi in range(2):
                                nc.tensor.matmul(
                                    otp[hi][:, plo:512],
                                    vau[:, ptb, 2 * hp + hi, :],
                                    pes[:, hi * 512 + plo:(hi + 1) * 512],
                                    start=(ppi == 0), stop=False,
                                    skip_group_check=True,
                                )
                        pv_prev = (pi, tb, lo, es)
                    ppi, ptb, plo, pes = pv_prev
                    for hi in range(2):
                        nc.tensor.matmul(
                            otp[hi][:, plo:512], vau[:, ptb, 2 * hp + hi, :],
                            pes[:, hi * 512 + plo:(hi + 1) * 512],
                            start=(ppi == 0), stop=True,
                            skip_group_check=True,
                        )
                    if sc == 3 and hp == 1:
                        # tail: start the ln chain straight off PSUM so it
                        # overlaps the accumulator drain
                        emit_ln_psum(otp, lnr_sh)
                    # drain O accumulators to SBUF promptly so the two PSUM
                    # banks recycle for the next head-pair; the two copies
                    # run on different engines so the drain latency halves.
                    osb = [op_.tile([65, 512], DT, tag="osb", name=f"osb{hi}")
                           for hi in range(2)]
                    with nc.allow_low_precision(reason="pre-normalize O"):
                        nc.vector.tensor_copy(osb[0][:], otp[0][:])
                        nc.scalar.copy(osb[1][:], otp[1][:])
                    osb_h[hp] = (osb, lnr_sh)

                osb_prev[(sc, 0)] = osb_h[0]
                if sc == 3:
                    # ln already emitted off PSUM at the drain above; heats
                    # keep the PE fed while the ACT chain completes.
                    heat(n=20)
                    emit_bcmult(3, 1, osb_h[1][0], lnr_sh)
                    emit_wo(3)
                else:
                    pend_norm = (sc, osb_h[1])

    if legalize:
        _legalize_waits(nc)
    return nc


def _legalize_waits(nc, max_waits=1):
    """Split >max_waits sync waits onto preceding same-engine NoOps
    (several instruction encodings only have one sync-wait slot)."""
    for fn in nc.m.functions:
        for bb in fn.blocks:
            new_insts = []
            for inst in bb.instructions:
                si = inst.sync_info
                waits = list(si.on_wait) if si is not None and si.on_wait else []
                if len(waits) > max_waits:
                    carry, keep = waits[:-max_waits], waits[-max_waits:]
                    for i, w in enumerate(carry):
                        new_insts.append(mybir.InstNoOp(
                            name=f"{inst.name}_wsplit{i}",
                            engine=inst.engine,
                            bass_nofuse=True,
                            sync_info=mybir.SyncInfo(on_wait=[w], on_update=[]),
                        ))
                    si.on_wait = keep
                new_insts.append(inst)
            bb.instructions[:] = new_insts


def _host_constants():
    # RoPE channel permutation: row r (within a head, 0..63) holds source
    # channel d = 2*i + odd with i = 16*(r//32) + r%16, odd = (r%32)//16.
    r = np.arange(64)
    i_ = 16 * (r // 32) + (r % 16)
    odd = (r % 32) // 16
    dsrc = 2 * i_ + odd  # source channel per permuted row

    inv_freq = ROPE_BASE ** (-(i_.astype(np.float64)) * 2.0 / Dh)
    ang = np.arange(S, dtype=np.float64)[None, :] * inv_freq[:, None]  # [64, S]
    cos64 = np.cos(ang)
    sin64 = np.sin(ang) * np.where(odd == 0, -1.0, 1.0)[:, None]
    cosd = np.tile(cos64, (2, 1)).astype(DT_NP)
    sins = np.tile(sin64, (2, 1)).astype(DT_NP)

    t = np.arange(128)[:, None]
    s = np.arange(128)[None, :]
    trim = (t <= s).astype(DT_NP)

    sel2 = np.zeros((33, 128), np.float32)
    sel2[0, 0:64] = 1
    sel2[32, 64:128] = 1
    return dsrc, cosd, sins, trim, sel2


def _wlay(w, dt=None):  # [E, HD] -> [p, ko, m] contiguous
    return np.ascontiguousarray(w.reshape(8, 128, HD).transpose(1, 0, 2)).astype(dt or DT_NP)


def _wolay(w):  # [HD, E] -> [p, ko, e] contiguous
    return np.ascontiguousarray(w.reshape(2, 128, E).transpose(1, 0, 2)).astype(DT_NP)


_CACHE = {}


def _run(inputs, trace=False):
    if "nc" not in _CACHE:
        _CACHE["nc"] = _build_program()
        _CACHE["consts"] = _host_constants()
    nc = _CACHE["nc"]
    dsrc, cosd, sins, trim, sel2 = _CACHE["consts"]

    x = np.ascontiguousarray(np.asarray(inputs["x"]), dtype=np.float32)
    W_q = np.asarray(inputs["W_q"], dtype=np.float32)
    W_k = np.asarray(inputs["W_k"], dtype=np.float32)
    W_v = np.asarray(inputs["W_v"], dtype=np.float32)
    W_o = np.asarray(inputs["W_o"], dtype=np.float32)

    # [p, sc, eo, s] so each chunk DMA is contiguous per partition
    xTc = [np.ascontiguousarray(
        x[b].reshape(NCHUNK, 512, 8, 128).transpose(3, 0, 2, 1))
        for b in range(B)]
    xTb = [t.astype(DT_NP) for t in xTc]
    xT8 = [t.astype(F8_NP) for t in xTc]

    in_maps = []
    for c in range(8):
        b, g = divmod(c, 4)
        heads = np.arange(4 * g, 4 * g + 4)
        rows_qk = (heads[:, None] * 64 + dsrc[None, :]).reshape(-1)   # permuted
        rows_v = (heads[:, None] * 64 + np.arange(64)[None, :]).reshape(-1)
        in_maps.append({
            "xT": xTb[b],
            "xT8": xT8[b],
            "wq8": _wlay(W_q[rows_qk].T * WS, F8_NP),
            "wk8": _wlay(W_k[rows_qk].T * WS, F8_NP),
            "wv": _wlay(W_v[rows_v].T),
            "wo": _wolay(W_o[:, rows_v].T),
            "cosd": cosd, "sins": sins, "trim": trim, "sel2c": sel2,
            "lnz": np.zeros((33, 512), np.float32),
        })

    res = bass_utils.run_bass_kernel_spmd(
        nc, in_maps, core_ids=list(range(8)), trace=trace,
    )
    out = np.zeros((B, S, E), np.float32)
    for c in range(8):
        out[c // 4] += res.results[c]["y"].astype(np.float32)
    return out, res


def kernel(**inputs):
    out, _ = _run(inputs, trace=False)
    return out


# revision 61
# speedup vs baseline: 1.0327x; 1.0261x over previous
"""Multi-head causal attention with RoPE on 8 TRN2 NeuronCores.

Sharding: batch (2) x head-groups (4 of 4 heads) -> 8 cores; host sums the
4 per-batch partial y's.

Per core, a software-pipelined stream over 512-row s-chunks:
- Q/K projections are fp8-e4m3 DoubleRow matmuls (2x PE throughput over the
  K=1024 contraction; host pre-casts x and 32-scaled W_q/W_k to fp8, the
  1024x score growth folds into the softmax exp scale). V projection,
  scores, PV and W_o stay bf16: fp8 on any of those overflows the 2e-2
  error gate, and scores/PV are PSUM-write-port bound at Dh=64 anyway.
- Transposed scores S^T = Kr @ Qr^T with both heads of a pair in one
  [128,1024] PSUM tile; causal 128-block skip with column-subrange matmuls
  and a post-exp triangle mask on diagonal blocks; PV with a ones-column on
  V accumulates the softmax denominator; PV trails the score stream by one
  block so the in-order PE never waits on an exp.
- Everything non-attention is woven between attention blocks, never in
  bursts: next-chunk x DMA + V projection into hp0, next-chunk Q/K
  projection+RoPE (quarter pieces: the mb0 half feeds hp0 scores, mb1 can
  land late) and prev-chunk W_o (one s-block at a time) into hp1, and the
  diagonal t-blocks are interleaved among full blocks so the cheap ones
  never bunch up.
- The softmax normalize is split into stages emitted far apart so its
  cross-engine chain never head-of-line-blocks an engine queue: ACT ln of
  the two denominator rows into a shared f32r tile, a selector matmul
  broadcasting them 2 blocks later, one wide exp(-x), then DVE scales into
  ot. W_o consumes ot a half-chunk behind.
- bf16 throughout with f32 PSUM accumulation; fp8 only where noted.
"""
import os
import sys

sys.path.insert(0, "/opt/trn_rl_repo")

import ml_dtypes
import numpy as np

import concourse.bass as bass
import concourse.mybir as mybir
import concourse.tile as tile
from concourse import bass_utils

F32 = mybir.dt.float32
BF16 = mybir.dt.bfloat16
F8 = mybir.dt.float8e4
F32R = mybir.dt.float32r

DT = BF16
DT_NP = ml_dtypes.bfloat16
F8_NP = ml_dtypes.float8_e4m3fn

B, S, E, H, Dh = 2, 2048, 1024, 16, 64
HG = 4            # heads per core
HD = HG * Dh      # 256 output channels per core
SCALE = float(1.0 / np.sqrt(np.float32(1024.0)))
WS = 32.0         # host scale on W_q/W_k so fp8 sees ~unit-variance weights
SCALE_QK = SCALE / (WS * WS)
ROPE_BASE = 10000.0
NCHUNK = S // 512     # 4 s-chunks of 512
NTB = S // 128        # 16 t-blocks of 128
SHUF16 = list(range(16, 32)) + list(range(0, 16))

Exp = mybir.ActivationFunctionType.Exp
Ln = mybir.ActivationFunctionType.Ln
MUL = mybir.AluOpType.mult
ADD = mybir.AluOpType.add
DR = mybir.MatmulPerfMode.DoubleRow


def _build_program(legalize=True):
    nc = bass.Bass("TRN2", target_bir_lowering=False, debug=False)

    xT = nc.dram_tensor("xT", [128, NCHUNK, 8, 512], DT, kind="ExternalInput")
    xT8 = nc.dram_tensor("xT8", [128, NCHUNK, 8, 512], F8, kind="ExternalInput")
    wq8 = nc.dram_tensor("wq8", [128, 8, HD], F8, kind="ExternalInput")
    wk8 = nc.dram_tensor("wk8", [128, 8, HD], F8, kind="ExternalInput")
    wv = nc.dram_tensor("wv", [128, 8, HD], DT, kind="ExternalInput")
    wo = nc.dram_tensor("wo", [128, 2, E], DT, kind="ExternalInput")
    cosd = nc.dram_tensor("cosd", [128, S], DT, kind="ExternalInput")
    sins = nc.dram_tensor("sins", [128, S], DT, kind="ExternalInput")
    trim = nc.dram_tensor("trim", [128, 128], DT, kind="ExternalInput")
    sel2c = nc.dram_tensor("sel2c", [33, 128], F32R, kind="ExternalInput")
    lnz = nc.dram_tensor("lnz", [33, 512], F32R, kind="ExternalInput")
    y = nc.dram_tensor("y", [S, E], DT, kind="ExternalOutput")

    with tile.TileContext(nc) as tc:
        with (
            tc.tile_pool(name="persist", bufs=1) as pp,
            tc.tile_pool(name="xchunks", bufs=3) as xp,
            tc.tile_pool(name="x8chunks", bufs=3) as xp8,
            tc.tile_pool(name="ropetmp", bufs=2) as rt,
            tc.tile_pool(name="att_es", bufs=6) as ep,
            tc.tile_pool(name="att_row", bufs=2) as rp,
            tc.tile_pool(name="osb", bufs=4) as op_,
            tc.tile_pool(name="ystg", bufs=2) as yp,
            tc.tile_pool(name="ps_big", bufs=3, space="PSUM") as psB,
            tc.tile_pool(name="ps_ot", bufs=2, space="PSUM") as psO,
        ):
            # ---- persistent tensors ----
            # Qr^T zero-padded per head half: qz[:, hi, blk, s] has rows of
            # head 2*blk+hi live and the other 64 rows zero, so the score
            # contraction runs over the full 128 partitions.
            qz = pp.tile([128, 2, 2, S], DT)
            krt = pp.tile([128, 2, S], DT)   # Kr^T
            vau = pp.tile([128, NTB, HG, 65], DT)  # V + ones col per (tb, h)
            ot = pp.tile([128, 2, S], DT)    # O^T normalized

            # stationary for PE-warming matmuls
            hW = pp.tile([128, 128], DT)
            nc.vector.memset(hW[:], 1.0)
            wrm = pp.tile([1, 16], DT)
            # single shared ln-rows tile: its write (pi2) / broadcast-read
            # (pi6) windows alternate without overlap; zero rows once so the
            # selector matmul never reads uninitialized memory
            lnr_sh = pp.tile([33, 512], F32R)

            def heat(n=10):
                # full-array 128x128 matmuls to trip the HAM activity window
                # back to K=8/8. Scratch lands in a big-ring PSUM slot whose
                # next real matmul uses start=True and overwrites it.
                htile = psB.tile([128, 128], F32, tag="big", name="heat")
                for _ in range(n):
                    nc.tensor.matmul(htile[:], hW[:], hW[:],
                                     start=True, stop=True)

            # Initial loads fan out over both hardware DMA queues so the
            # first-needed tensors don't wait behind the rest.
            # ordered by first use: Q-proj inputs, RoPE tables, K, then
            # the attention-phase tensors (V/x bf16, masks, W_o)
            wq8_sb = pp.tile([128, 8, HD], F8)
            nc.sync.dma_start(wq8_sb[:], wq8.ap())
            xc80 = xp8.tile([128, 8, 512], F8, tag="xc8", name="xc80")
            nc.scalar.dma_start(xc80[:], xT8.ap()[:, 0, :, :])
            cos_sb = pp.tile([128, S], DT)
            nc.sync.dma_start(cos_sb[:], cosd.ap())
            sin_sb = pp.tile([128, S], DT)
            nc.scalar.dma_start(sin_sb[:], sins.ap())
            wk8_sb = pp.tile([128, 8, HD], F8)
            nc.sync.dma_start(wk8_sb[:], wk8.ap())
            wv_sb = pp.tile([128, 8, HD], DT)
            nc.scalar.dma_start(wv_sb[:], wv.ap())
            xc0 = xp.tile([128, 8, 512], DT, tag="xc", name="xc0")
            nc.sync.dma_start(xc0[:, 0:4, :], xT.ap()[:, 0, 0:4, :])
            nc.scalar.dma_start(xc0[:, 4:8, :], xT.ap()[:, 0, 4:8, :])
            tri_sb = pp.tile([128, 128], DT)
            nc.scalar.dma_start(tri_sb[:], trim.ap())
            sel2_sb = pp.tile([33, 128], F32R)
            nc.scalar.dma_start(sel2_sb[:], sel2c.ap())
            wo_sb = pp.tile([128, 2, E], DT)
            nc.scalar.dma_start(wo_sb[:], wo.ap())

            nc.scalar.dma_start(lnr_sh[:], lnz.ap())
            nc.gpsimd.memset(qz[64:128, 0, :, :], 0.0)
            nc.gpsimd.memset(qz[0:64, 1, :, :], 0.0)
            nc.gpsimd.memset(vau[:, :, :, 64:65], 1.0)

            # preload the ACT exp/ln tables while DMAs stream (first real
            # activation otherwise eats a 1.3us ACT_TABLE_LOAD); writes to a
            # dedicated scratch so nothing waits on the slow first GpSimd op
            with nc.allow_low_precision(reason="ACT table warm"):
                nc.scalar.activation(wrm[:], hW[0:1, 0:16], Exp, bias=0.0, scale=1.0)
                nc.scalar.activation(wrm[:], hW[0:1, 0:16], Ln, bias=0.0, scale=1.0)

            # warm the PE during the initial DMA streams
            heat(n=58)

            def emit_ln(osb, lnr):
                # ln of the two softmax denominator rows into adjacent
                # partitions of one lnr tile (from the drained osb rows)
                for hi in range(2):
                    nc.scalar.activation(lnr[32 * hi:32 * hi + 1, :],
                                         osb[hi][64:65, :], Ln, bias=0.0, scale=1.0)

            def emit_ln_psum(otp, lnr):
                # tail path: ln straight off the PSUM accumulator rows
                for hi in range(2):
                    nc.scalar.activation(lnr[32 * hi:32 * hi + 1, :],
                                         otp[hi][64:65, :], Ln, bias=0.0, scale=1.0)

            def emit_bcmult(sc, hp, osb, lnr):
                # broadcast the two ln rows with one selector matmul, one
                # full-width exp(-x) recovers the reciprocals, then DVE
                # scales the O rows into ot. Emitted several blocks after
                # emit_ln so the bc matmul never head-of-line-blocks the
                # in-order PE queue.
                ss = slice(sc * 512, (sc + 1) * 512)
                bc = psB.tile([128, 512], F32, tag="big", name="bc")
                nc.tensor.matmul(bc[:], sel2_sb[:], lnr[:], start=True, stop=True)
                ew = rp.tile([64, 2, 512], DT, tag="ew", name="ew")
                with nc.allow_low_precision(reason="recip broadcast"):
                    for hi in range(2):
                        nc.scalar.activation(ew[:, hi, :],
                                             bc[hi * 64:(hi + 1) * 64, :],
                                             Exp, bias=0.0, scale=-1.0)
                for hi in range(2):
                    with nc.allow_low_precision(reason="normalized O rows"):
                        nc.vector.tensor_tensor(ot[hi * 64:(hi + 1) * 64, hp, ss],
                                                osb[hi][0:64, :],
                                                ew[:, hi, :], MUL)

            def emit_wo_sbl(sc, sbl, tail_heat=False):
                # W_o partials for one 128-row s-block of chunk sc; both
                # 512-col E halves accumulate in one [128,1024] PSUM pair.
                sb_i = sc * 4 + sbl
                tsl = slice(sb_i * 128, (sb_i + 1) * 128)
                py = psB.tile([128, 1024], F32, tag="big", name="py")
                for ec in range(2):
                    for blk in range(2):
                        nc.tensor.matmul(
                            py[:, ec * 512:(ec + 1) * 512], ot[:, blk, tsl],
                            wo_sb[:, blk, ec * 512:(ec + 1) * 512],
                            start=(blk == 0), stop=(blk == 1),
                        )
                ystg = yp.tile([128, E], DT, tag="y")
                with nc.allow_low_precision(reason="partial sum staging"):
                    if tail_heat:
                        # tail: alternate engines so the four stagings
                        # don't serialize on one queue
                        if sbl % 2 == 0:
                            nc.scalar.copy(ystg[:], py[:])
                        else:
                            nc.vector.tensor_copy(ystg[:], py[:])
                        heat(n=2)
                    else:
                        nc.vector.tensor_copy(ystg[:], py[:])
                nc.sync.dma_start(y.ap()[tsl, :], ystg[:])

            def emit_wo(sc):
                for sbl in range(4):
                    emit_wo_sbl(sc, sbl, tail_heat=(sc == 3))

            pend_norm = None  # (sc, osb) for hp=1, normalized next chunk
            osb_prev = {}

            def dma_chunk(sc):
                xc = xp.tile([128, 8, 512], DT, tag="xc", name=f"xc{sc}")
                nc.sync.dma_start(xc[:], xT.ap()[:, sc, :, :])
                xc8 = xp8.tile([128, 8, 512], F8, tag="xc8", name=f"xc8{sc}")
                nc.scalar.dma_start(xc8[:], xT8.ap()[:, sc, :, :])
                return xc, xc8

            def vproj(sc, xc, tbl):
                # V rows for one of chunk sc's 4 new t-blocks; only the
                # diagonal PVs at the end of this chunk's tb loop need them.
                tb = sc * 4 + tbl
                pv = psB.tile([128, 256], F32, tag="big", name="pv")
                for e in range(8):
                    nc.tensor.matmul(
                        pv[:], xc[:, e, tbl * 128:(tbl + 1) * 128],
                        wv_sb[:, e, :], start=(e == 0), stop=(e == 7),
                    )
                with nc.allow_low_precision(reason="rounded matmul input"):
                    nc.vector.tensor_copy(
                        vau[:, tb, :, 0:64],
                        pv[:].rearrange("p (h d) -> p h d", d=64),
                    )

            def qkproj_mb(sc, xc8, w_sb, dst, mb):
                # one 128-chan half of a fp8 DoubleRow projection + its RoPE
                # for chunk sc; dst=None -> qz (zero-padded halves), else
                # krt. mb0 feeds blk0 (hp0) scores, mb1 feeds hp1 -- so mb1
                # pieces can be emitted much later.
                ss = slice(sc * 512, (sc + 1) * 512)
                pq = psB.tile([128, 512], F32, tag="big", name="pq")
                for ktp in range(4):
                    nc.tensor.matmul(
                        pq[:],
                        w_sb[:, 2 * ktp:2 * ktp + 2, mb * 128:(mb + 1) * 128],
                        xc8[:, 2 * ktp:2 * ktp + 2, :],
                        start=(ktp == 0), stop=(ktp == 3),
                        perf_mode=DR,
                    )
                a = rt.tile([128, 512], DT, tag="a")
                with nc.allow_low_precision(reason="rounded matmul input"):
                    nc.vector.tensor_copy(a[:], pq[:])
                bsh = rt.tile([128, 512], DT, tag="b")
                nc.vector.stream_shuffle(bsh[:], a[:], SHUF16)
                t1 = rt.tile([128, 512], DT, tag="t1")
                t2 = rt.tile([128, 512], DT, tag="t2")
                with nc.allow_low_precision(reason="rounded matmul input"):
                    nc.vector.tensor_tensor(t1[:], bsh[:], sin_sb[:, ss], MUL)
                    nc.vector.tensor_tensor(t2[:], a[:], cos_sb[:, ss], MUL)
                    if dst is None:  # Q: split into zero-padded halves
                        nc.vector.tensor_tensor(
                            qz[0:64, 0, mb, ss], t2[0:64, :], t1[0:64, :], ADD)
                        nc.vector.tensor_tensor(
                            qz[64:128, 1, mb, ss], t2[64:128, :], t1[64:128, :], ADD)
                    else:
                        nc.vector.tensor_tensor(
                            dst[:, mb, ss], t2[:, :], t1[:, :], ADD)

            # ---- chunk 0 prologue: mb0 halves first so hp0 scores can
            # start while the mb1 halves are still in flight.
            xcs = {0: (xc0, xc80)}
            qkproj_mb(0, xc80, wq8_sb, None, 0)
            qkproj_mb(0, xc80, wk8_sb, krt, 0)
            # chunk 0's V projection runs here, filling the PE while the DVE
            # RoPE chains for the mb0 halves drain; its vau tiles are only
            # needed once attention starts.
            for tbl in range(4):
                vproj(0, xc0, tbl)
            qkproj_mb(0, xc80, wq8_sb, None, 1)
            qkproj_mb(0, xc80, wk8_sb, krt, 1)


            for sc in range(NCHUNK):
                ntb = 4 * sc + 4
                xc, xc8 = xcs[sc]

                # ---- attention for this chunk, one head-pair at a time,
                # with next-chunk projection, V, W_o and normalize chains
                # woven between attention blocks so no engine sees a burst.
                osb_h = [None, None]
                fulls = list(range(4 * sc))
                diags = [4 * sc + i for i in range(4)]
                if sc == 0:
                    order = diags
                else:
                    k = len(fulls) // 4
                    order = []
                    fi = 0
                    for i, d in enumerate(diags):
                        take = max(2, k) if i == 0 else k
                        take = min(take, len(fulls) - fi)
                        order += fulls[fi:fi + take]
                        fi += take
                        order.append(d)
                    order += fulls[fi:]
                qk_step = max(ntb // 4, 1)

                for hp in range(2):
                    otp = [psO.tile([65, 512], F32, tag="ot", name=f"otp{hi}")
                           for hi in range(2)]
                    blk = hp
                    wo_step = max(ntb // 4, 2)
                    pv_fifo = []  # (pi, tb, lo, es), PV trails scores by 2
                    for pi, tb in enumerate(order):
                        m = tb - 4 * sc
                        lo = 128 * max(m, 0)  # diag: skip cols left of block
                        if hp == 0:
                            if pi == 0 and sc < 3:
                                xcs[sc + 1] = dma_chunk(sc + 1)
                            if pi < 4 and sc > 0:
                                vproj(sc, xc, pi)
                            if pi == 1 and sc == 1:
                                # chunk 0 hp0's bc didn't fit in its 4-block
                                # hp1 loop; runs here
                                emit_bcmult(0, 0, *osb_prev[(0, 0)])
                            if pi == 2 and pend_norm is not None:
                                emit_ln(pend_norm[1][0], pend_norm[1][1])
                            if pi == 6 and pend_norm is not None:
                                emit_bcmult(pend_norm[0], 1, *pend_norm[1])
                                pend_norm = None
                        else:
                            if pi == 2:
                                # deferred normalize of this chunk's hp=0
                                emit_ln(osb_h[0][0], osb_h[0][1])
                            if pi == 6:
                                emit_bcmult(sc, 0, *osb_h[0])
                            if (sc > 0 and pi >= 1
                                    and (pi - 1) % wo_step == 0
                                    and (pi - 1) // wo_step < 4):
                                # W_o of the previous chunk, one s-block at
                                # a time
                                emit_wo_sbl(sc - 1, (pi - 1) // wo_step)
                            if sc < 3 and pi % qk_step == 0 and pi // qk_step < 4:
                                # next chunk's projection, one (proj, mb)
                                # piece at a time: Q-mb0, K-mb0, Q-mb1, K-mb1
                                j = pi // qk_step
                                w_sb_n = (wq8_sb, wk8_sb)[j % 2]
                                dst_n = (None, krt)[j % 2]
                                qkproj_mb(sc + 1, xcs[sc + 1][1], w_sb_n,
                                          dst_n, j // 2)
                        pss = psB.tile([128, 1024], F32, tag="big", name="pss")
                        ps3 = pss[:].rearrange("p (h s) -> p h s", h=2)
                        for hi in range(2):
                            nc.tensor.matmul(
                                ps3[:, hi, lo:512],
                                krt[:, blk, tb * 128:(tb + 1) * 128],
                                qz[:, hi, blk, sc * 512 + lo:(sc + 1) * 512],
                                start=True, stop=True,
                            )
                        es = ep.tile([128, 1024], DT, tag="es", name="es")
                        es3 = es[:].rearrange("p (h s) -> p h s", h=2)
                        with nc.allow_low_precision(reason="rounded matmul input"):
                            nc.scalar.activation(es3[:, :, lo:512], ps3[:, :, lo:512],
                                                 Exp, bias=0.0, scale=SCALE_QK)
                        if m >= 0:  # mask the diagonal 128-col triangle
                            trib = tri_sb[:].rearrange("p (o s) -> p o s", o=1).to_broadcast((128, 2, 128))
                            with nc.allow_low_precision(reason="rounded matmul input"):
                                nc.vector.tensor_tensor(
                                    es3[:, :, lo:lo + 128], es3[:, :, lo:lo + 128],
                                    trib, MUL)
                        # PV trails the score stream by two blocks: the PE
                        # always has scores queued ahead of a PV that may
                        # still be waiting on its exp.
                        pv_fifo.append((pi, tb, lo, es))
                        if len(pv_fifo) > 2:
                            ppi, ptb, plo, pes = pv_fifo.pop(0)
                            for hi in range(2):
                                nc.tensor.matmul(
                                    otp[hi][:, plo:512],
                                    vau[:, ptb, 2 * hp + hi, :],
                                    pes[:, hi * 512 + plo:(hi + 1) * 512],
                                    start=(ppi == 0), stop=False,
                                    skip_group_check=True,
                                )
                    for qi, (ppi, ptb, plo, pes) in enumerate(pv_fifo):
                        for hi in range(2):
                            nc.tensor.matmul(
                                otp[hi][:, plo:512], vau[:, ptb, 2 * hp + hi, :],
                                pes[:, hi * 512 + plo:(hi + 1) * 512],
                                start=(ppi == 0), stop=(qi == len(pv_fifo) - 1),
                                skip_group_check=True,
                            )
                    if sc == 3 and hp == 1:
                        # tail: start the ln chain straight off PSUM so it
                        # overlaps the accumulator drain
                        emit_ln_psum(otp, lnr_sh)
                    # drain O accumulators to SBUF promptly so the two PSUM
                    # banks recycle for the next head-pair; the two copies
                    # run on different engines so the drain latency halves.
                    osb = [op_.tile([65, 512], DT, tag="osb", name=f"osb{hi}")
                           for hi in range(2)]
                    with nc.allow_low_precision(reason="pre-normalize O"):
                        nc.vector.tensor_copy(osb[0][:], otp[0][:])
                        nc.scalar.copy(osb[1][:], otp[1][:])
                    osb_h[hp] = (osb, lnr_sh)

                osb_prev[(sc, 0)] = osb_h[0]
                if sc == 3:
                    # ln already emitted off PSUM at the drain above; heats
                    # keep the PE fed while the ACT chain completes.
                    heat(n=20)
                    emit_bcmult(3, 1, osb_h[1][0], lnr_sh)
                    emit_wo(3)
                else:
                    pend_norm = (sc, osb_h[1])

    if legalize:
        _legalize_waits(nc)
    return nc


def _legalize_waits(nc, max_waits=1):
    """Split >max_waits sync waits onto preceding same-engine NoOps
    (several instruction encodings only have one sync-wait slot)."""
    for fn in nc.m.functions:
        for bb in fn.blocks:
            new_insts = []
            for inst in bb.instructions:
                si = inst.sync_info
                waits = list(si.on_wait) if si is not None and si.on_wait else []
                if len(waits) > max_waits:
                    carry, keep = waits[:-max_waits], waits[-max_waits:]
                    for i, w in enumerate(carry):
                        new_insts.append(mybir.InstNoOp(
                            name=f"{inst.name}_wsplit{i}",
                            engine=inst.engine,
                            bass_nofuse=True,
                            sync_info=mybir.SyncInfo(on_wait=[w], on_update=[]),
                        ))
                    si.on_wait = keep
                new_insts.append(inst)
            bb.instructions[:] = new_insts


def _host_constants():
    # RoPE channel permutation: row r (within a head, 0..63) holds source
    # channel d = 2*i + odd with i = 16*(r//32) + r%16, odd = (r%32)//16.
    r = np.arange(64)
    i_ = 16 * (r // 32) + (r % 16)
    odd = (r % 32) // 16
    dsrc = 2 * i_ + odd  # source channel per permuted row

    inv_freq = ROPE_BASE ** (-(i_.astype(np.float64)) * 2.0 / Dh)
    ang = np.arange(S, dtype=np.float64)[None, :] * inv_freq[:, None]  # [64, S]
    cos64 = np.cos(ang)
    sin64 = np.sin(ang) * np.where(odd == 0, -1.0, 1.0)[:, None]
    cosd = np.tile(cos64, (2, 1)).astype(DT_NP)
    sins = np.tile(sin64, (2, 1)).astype(DT_NP)

    t = np.arange(128)[:, None]
    s = np.arange(128)[None, :]
    trim = (t <= s).astype(DT_NP)

    sel2 = np.zeros((33, 128), np.float32)
    sel2[0, 0:64] = 1
    sel2[32, 64:128] = 1
    return dsrc, cosd, sins, trim, sel2


def _wlay(w, dt=None):  # [E, HD] -> [p, ko, m] contiguous
    return np.ascontiguousarray(w.reshape(8, 128, HD).transpose(1, 0, 2)).astype(dt or DT_NP)


def _wolay(w):  # [HD, E] -> [p, ko, e] contiguous
    return np.ascontiguousarray(w.reshape(2, 128, E).transpose(1, 0, 2)).astype(DT_NP)


_CACHE = {}


def _run(inputs, trace=False):
    if "nc" not in _CACHE:
        _CACHE["nc"] = _build_program()
        _CACHE["consts"] = _host_constants()
    nc = _CACHE["nc"]
    dsrc, cosd, sins, trim, sel2 = _CACHE["consts"]

    x = np.ascontiguousarray(np.asarray(inputs["x"]), dtype=np.float32)
    W_q = np.asarray(inputs["W_q"], dtype=np.float32)
    W_k = np.asarray(inputs["W_k"], dtype=np.float32)
    W_v = np.asarray(inputs["W_v"], dtype=np.float32)
    W_o = np.asarray(inputs["W_o"], dtype=np.float32)

    # [p, sc, eo, s] so each chunk DMA is contiguous per partition
    xTc = [np.ascontiguousarray(
        x[b].reshape(NCHUNK, 512, 8, 128).transpose(3, 0, 2, 1))
        for b in range(B)]
    xTb = [t.astype(DT_NP) for t in xTc]
    xT8 = [t.astype(F8_NP) for t in xTc]

    in_maps = []
    for c in range(8):
        b, g = divmod(c, 4)
        heads = np.arange(4 * g, 4 * g + 4)
        rows_qk = (heads[:, None] * 64 + dsrc[None, :]).reshape(-1)   # permuted
        rows_v = (heads[:, None] * 64 + np.arange(64)[None, :]).reshape(-1)
        in_maps.append({
            "xT": xTb[b],
            "xT8": xT8[b],
            "wq8": _wlay(W_q[rows_qk].T * WS, F8_NP),
            "wk8": _wlay(W_k[rows_qk].T * WS, F8_NP),
            "wv": _wlay(W_v[rows_v].T),
            "wo": _wolay(W_o[:, rows_v].T),
            "cosd": cosd, "sins": sins, "trim": trim, "sel2c": sel2,
            "lnz": np.zeros((33, 512), np.float32),
        })

    res = bass_utils.run_bass_kernel_spmd(
        nc, in_maps, core_ids=list(range(8)), trace=trace,
    )
    out = np.zeros((B, S, E), np.float32)
    for c in range(8):
        out[c // 4] += res.results[c]["y"].astype(np.float32)
    return out, res


def kernel(**inputs):
    out, _ = _run(inputs, trace=False)
    return out


# revision 63
# speedup vs baseline: 1.0464x; 1.0132x over previous
"""Multi-head causal attention with RoPE on 8 TRN2 NeuronCores.

Sharding: batch (2) x head-groups (4 of 4 heads) -> 8 cores; host sums the
4 per-batch partial y's.

Per core, a software-pipelined stream over 512-row s-chunks:
- Q/K projections are fp8-e4m3 DoubleRow matmuls (2x PE throughput over the
  K=1024 contraction; host pre-casts x and 32-scaled W_q/W_k to fp8, the
  1024x score growth folds into the softmax exp scale). V projection,
  scores, PV and W_o stay bf16: fp8 on any of those overflows the 2e-2
  error gate, and scores/PV are PSUM-write-port bound at Dh=64 anyway.
- Transposed scores S^T = Kr @ Qr^T with both heads of a pair in one
  [128,1024] PSUM tile; causal 128-block skip with column-subrange matmuls
  and a post-exp triangle mask on diagonal blocks; PV with a ones-column on
  V accumulates the softmax denominator; PV trails the score stream by two
  blocks so the in-order PE never waits on an exp.
- Everything non-attention is woven between attention blocks, never in
  bursts: next-chunk x DMA + V projection into hp0, next-chunk Q/K
  projection+RoPE (quarter pieces: the mb0 half feeds hp0 scores, mb1 can
  land late) and prev-chunk W_o (one s-block at a time) into hp1, and the
  diagonal t-blocks are interleaved among full blocks so the cheap ones
  never bunch up.
- The softmax normalize is split into stages emitted far apart so its
  cross-engine chain never head-of-line-blocks an engine queue: ACT ln of
  the two denominator rows into a shared f32r tile, a selector matmul
  broadcasting them 2 blocks later, one wide exp(-x), then DVE scales into
  ot. W_o consumes ot a half-chunk behind.
- bf16 throughout with f32 PSUM accumulation; fp8 only where noted.
"""
import os
import sys

sys.path.insert(0, "/opt/trn_rl_repo")

import ml_dtypes
import numpy as np

import concourse.bass as bass
import concourse.mybir as mybir
import concourse.tile as tile
from concourse import bass_utils

F32 = mybir.dt.float32
BF16 = mybir.dt.bfloat16
F8 = mybir.dt.float8e4
F32R = mybir.dt.float32r

DT = BF16
DT_NP = ml_dtypes.bfloat16
F8_NP = ml_dtypes.float8_e4m3fn

B, S, E, H, Dh = 2, 2048, 1024, 16, 64
HG = 4            # heads per core
HD = HG * Dh      # 256 output channels per core
SCALE = float(1.0 / np.sqrt(np.float32(1024.0)))
WS = 32.0         # host scale on W_q/W_k so fp8 sees ~unit-variance weights
SCALE_QK = SCALE / (WS * WS)
ROPE_BASE = 10000.0
NCHUNK = S // 512     # 4 s-chunks of 512
NTB = S // 128        # 16 t-blocks of 128
SHUF16 = list(range(16, 32)) + list(range(0, 16))

Exp = mybir.ActivationFunctionType.Exp
Ln = mybir.ActivationFunctionType.Ln
MUL = mybir.AluOpType.mult
ADD = mybir.AluOpType.add
DR = mybir.MatmulPerfMode.DoubleRow


def _build_program(legalize=True):
    nc = bass.Bass("TRN2", target_bir_lowering=False, debug=False)

    xT = nc.dram_tensor("xT", [128, NCHUNK, 8, 512], DT, kind="ExternalInput")
    xT8 = nc.dram_tensor("xT8", [128, NCHUNK, 8, 512], F8, kind="ExternalInput")
    wq8 = nc.dram_tensor("wq8", [128, 8, HD], F8, kind="ExternalInput")
    wk8 = nc.dram_tensor("wk8", [128, 8, HD], F8, kind="ExternalInput")
    wv = nc.dram_tensor("wv", [128, 8, HD], DT, kind="ExternalInput")
    wo = nc.dram_tensor("wo", [128, 2, E], DT, kind="ExternalInput")
    cosd = nc.dram_tensor("cosd", [128, S], DT, kind="ExternalInput")
    sins = nc.dram_tensor("sins", [128, S], DT, kind="ExternalInput")
    trim = nc.dram_tensor("trim", [128, 128], DT, kind="ExternalInput")
    sel2c = nc.dram_tensor("sel2c", [33, 128], F32R, kind="ExternalInput")
    lnz = nc.dram_tensor("lnz", [33, 512], F32R, kind="ExternalInput")
    y = nc.dram_tensor("y", [S, E], DT, kind="ExternalOutput")

    with tile.TileContext(nc) as tc:
        with (
            tc.tile_pool(name="persist", bufs=1) as pp,
            tc.tile_pool(name="xchunks", bufs=3) as xp,
            tc.tile_pool(name="x8chunks", bufs=3) as xp8,
            tc.tile_pool(name="ropetmp", bufs=3) as rt,
            tc.tile_pool(name="att_es", bufs=8) as ep,
            tc.tile_pool(name="att_row", bufs=2) as rp,
            tc.tile_pool(name="osb", bufs=4) as op_,
            tc.tile_pool(name="ystg", bufs=2) as yp,
            tc.tile_pool(name="ps_big", bufs=3, space="PSUM") as psB,
            tc.tile_pool(name="ps_ot", bufs=2, space="PSUM") as psO,
        ):
            # ---- persistent tensors ----
            # Qr^T zero-padded per head half: qz[:, hi, blk, s] has rows of
            # head 2*blk+hi live and the other 64 rows zero, so the score
            # contraction runs over the full 128 partitions.
            qz = pp.tile([128, 2, 2, S], DT)
            krt = pp.tile([128, 2, S], DT)   # Kr^T
            vau = pp.tile([128, NTB, HG, 65], DT)  # V + ones col per (tb, h)
            ot = pp.tile([128, 2, S], DT)    # O^T normalized

            # stationary for PE-warming matmuls
            hW = pp.tile([128, 128], DT)
            nc.vector.memset(hW[:], 1.0)
            wrm = pp.tile([1, 16], DT)
            # single shared ln-rows tile: its write (pi2) / broadcast-read
            # (pi6) windows alternate without overlap; zero rows once so the
            # selector matmul never reads uninitialized memory
            lnr_sh = pp.tile([33, 512], F32R)

            def heat(n=10):
                # full-array 128x128 matmuls to trip the HAM activity window
                # back to K=8/8. Scratch lands in a big-ring PSUM slot whose
                # next real matmul uses start=True and overwrites it.
                htile = psB.tile([128, 128], F32, tag="big", name="heat")
                for _ in range(n):
                    nc.tensor.matmul(htile[:], hW[:], hW[:],
                                     start=True, stop=True)

            # Initial loads fan out over both hardware DMA queues so the
            # first-needed tensors don't wait behind the rest.
            # ordered by first use: Q-proj inputs, RoPE tables, K, then
            # the attention-phase tensors (V/x bf16, masks, W_o)
            wq8_sb = pp.tile([128, 8, HD], F8)
            nc.sync.dma_start(wq8_sb[:], wq8.ap())
            xc80 = xp8.tile([128, 8, 512], F8, tag="xc8", name="xc80")
            nc.scalar.dma_start(xc80[:], xT8.ap()[:, 0, :, :])
            cos_sb = pp.tile([128, S], DT)
            nc.sync.dma_start(cos_sb[:], cosd.ap())
            sin_sb = pp.tile([128, S], DT)
            nc.scalar.dma_start(sin_sb[:], sins.ap())
            wk8_sb = pp.tile([128, 8, HD], F8)
            nc.sync.dma_start(wk8_sb[:], wk8.ap())
            wv_sb = pp.tile([128, 8, HD], DT)
            nc.scalar.dma_start(wv_sb[:], wv.ap())
            xc0 = xp.tile([128, 8, 512], DT, tag="xc", name="xc0")
            nc.sync.dma_start(xc0[:, 0:4, :], xT.ap()[:, 0, 0:4, :])
            nc.scalar.dma_start(xc0[:, 4:8, :], xT.ap()[:, 0, 4:8, :])
            tri_sb = pp.tile([128, 128], DT)
            nc.scalar.dma_start(tri_sb[:], trim.ap())
            sel2_sb = pp.tile([33, 128], F32R)
            nc.scalar.dma_start(sel2_sb[:], sel2c.ap())
            wo_sb = pp.tile([128, 2, E], DT)
            nc.scalar.dma_start(wo_sb[:], wo.ap())

            nc.scalar.dma_start(lnr_sh[:], lnz.ap())
            nc.gpsimd.memset(qz[64:128, 0, :, :], 0.0)
            nc.gpsimd.memset(qz[0:64, 1, :, :], 0.0)
            nc.gpsimd.memset(vau[:, :, :, 64:65], 1.0)

            # preload the ACT exp/ln tables while DMAs stream (first real
            # activation otherwise eats a 1.3us ACT_TABLE_LOAD); writes to a
            # dedicated scratch so nothing waits on the slow first GpSimd op
            with nc.allow_low_precision(reason="ACT table warm"):
                nc.scalar.activation(wrm[:], hW[0:1, 0:16], Exp, bias=0.0, scale=1.0)
                nc.scalar.activation(wrm[:], hW[0:1, 0:16], Ln, bias=0.0, scale=1.0)

            # warm the PE during the initial DMA streams
            heat(n=58)

            def emit_ln(osb, lnr):
                # ln of the two softmax denominator rows into adjacent
                # partitions of one lnr tile (from the drained osb rows)
                for hi in range(2):
                    nc.scalar.activation(lnr[32 * hi:32 * hi + 1, :],
                                         osb[hi][64:65, :], Ln, bias=0.0, scale=1.0)

            def emit_ln_psum(otp, lnr):
                # tail path: ln straight off the PSUM accumulator rows
                for hi in range(2):
                    nc.scalar.activation(lnr[32 * hi:32 * hi + 1, :],
                                         otp[hi][64:65, :], Ln, bias=0.0, scale=1.0)

            def emit_bcmult(sc, hp, osb, lnr):
                # broadcast the two ln rows with one selector matmul, one
                # full-width exp(-x) recovers the reciprocals, then DVE
                # scales the O rows into ot. Emitted several blocks after
                # emit_ln so the bc matmul never head-of-line-blocks the
                # in-order PE queue.
                ss = slice(sc * 512, (sc + 1) * 512)
                bc = psB.tile([128, 512], F32, tag="big", name="bc")
                nc.tensor.matmul(bc[:], sel2_sb[:], lnr[:], start=True, stop=True)
                ew = rp.tile([64, 2, 512], DT, tag="ew", name="ew")
                with nc.allow_low_precision(reason="recip broadcast"):
                    for hi in range(2):
                        nc.scalar.activation(ew[:, hi, :],
                                             bc[hi * 64:(hi + 1) * 64, :],
                                             Exp, bias=0.0, scale=-1.0)
                for hi in range(2):
                    with nc.allow_low_precision(reason="normalized O rows"):
                        nc.vector.tensor_tensor(ot[hi * 64:(hi + 1) * 64, hp, ss],
                                                osb[hi][0:64, :],
                                                ew[:, hi, :], MUL)

            def emit_wo_sbl(sc, sbl, tail_heat=False):
                # W_o partials for one 128-row s-block of chunk sc; both
                # 512-col E halves accumulate in one [128,1024] PSUM pair.
                sb_i = sc * 4 + sbl
                tsl = slice(sb_i * 128, (sb_i + 1) * 128)
                py = psB.tile([128, 1024], F32, tag="big", name="py")
                for ec in range(2):
                    for blk in range(2):
                        nc.tensor.matmul(
                            py[:, ec * 512:(ec + 1) * 512], ot[:, blk, tsl],
                            wo_sb[:, blk, ec * 512:(ec + 1) * 512],
                            start=(blk == 0), stop=(blk == 1),
                        )
                ystg = yp.tile([128, E], DT, tag="y")
                with nc.allow_low_precision(reason="partial sum staging"):
                    if tail_heat:
                        # tail: alternate engines so the four stagings
                        # don't serialize on one queue
                        if sbl % 2 == 0:
                            nc.scalar.copy(ystg[:], py[:])
                        else:
                            nc.vector.tensor_copy(ystg[:], py[:])
                        heat(n=2)
                    else:
                        nc.vector.tensor_copy(ystg[:], py[:])
                nc.sync.dma_start(y.ap()[tsl, :], ystg[:])

            def emit_wo(sc):
                for sbl in range(4):
                    emit_wo_sbl(sc, sbl, tail_heat=(sc == 3))

            pend_norm = None  # (sc, osb) for hp=1, normalized next chunk
            osb_prev = {}

            def dma_chunk(sc):
                xc = xp.tile([128, 8, 512], DT, tag="xc", name=f"xc{sc}")
                nc.sync.dma_start(xc[:], xT.ap()[:, sc, :, :])
                xc8 = xp8.tile([128, 8, 512], F8, tag="xc8", name=f"xc8{sc}")
                nc.scalar.dma_start(xc8[:], xT8.ap()[:, sc, :, :])
                return xc, xc8

            def vproj(sc, xc, tbl):
                # V rows for one of chunk sc's 4 new t-blocks; only the
                # diagonal PVs at the end of this chunk's tb loop need them.
                tb = sc * 4 + tbl
                pv = psB.tile([128, 256], F32, tag="big", name="pv")
                for e in range(8):
                    nc.tensor.matmul(
                        pv[:], xc[:, e, tbl * 128:(tbl + 1) * 128],
                        wv_sb[:, e, :], start=(e == 0), stop=(e == 7),
                    )
                with nc.allow_low_precision(reason="rounded matmul input"):
                    nc.vector.tensor_copy(
                        vau[:, tb, :, 0:64],
                        pv[:].rearrange("p (h d) -> p h d", d=64),
                    )

            def qkproj_mb(sc, xc8, w_sb, dst, mb):
                # one 128-chan half of a fp8 DoubleRow projection + its RoPE
                # for chunk sc; dst=None -> qz (zero-padded halves), else
                # krt. mb0 feeds blk0 (hp0) scores, mb1 feeds hp1 -- so mb1
                # pieces can be emitted much later.
                ss = slice(sc * 512, (sc + 1) * 512)
                pq = psB.tile([128, 512], F32, tag="big", name="pq")
                for ktp in range(4):
                    nc.tensor.matmul(
                        pq[:],
                        w_sb[:, 2 * ktp:2 * ktp + 2, mb * 128:(mb + 1) * 128],
                        xc8[:, 2 * ktp:2 * ktp + 2, :],
                        start=(ktp == 0), stop=(ktp == 3),
                        perf_mode=DR,
                    )
                a = rt.tile([128, 512], DT, tag="a")
                with nc.allow_low_precision(reason="rounded matmul input"):
                    nc.vector.tensor_copy(a[:], pq[:])
                bsh = rt.tile([128, 512], DT, tag="b")
                nc.vector.stream_shuffle(bsh[:], a[:], SHUF16)
                t1 = rt.tile([128, 512], DT, tag="t1")
                t2 = rt.tile([128, 512], DT, tag="t2")
                with nc.allow_low_precision(reason="rounded matmul input"):
                    nc.vector.tensor_tensor(t1[:], bsh[:], sin_sb[:, ss], MUL)
                    nc.vector.tensor_tensor(t2[:], a[:], cos_sb[:, ss], MUL)
                    if dst is None:  # Q: split into zero-padded halves
                        nc.vector.tensor_tensor(
                            qz[0:64, 0, mb, ss], t2[0:64, :], t1[0:64, :], ADD)
                        nc.vector.tensor_tensor(
                            qz[64:128, 1, mb, ss], t2[64:128, :], t1[64:128, :], ADD)
                    else:
                        nc.vector.tensor_tensor(
                            dst[:, mb, ss], t2[:, :], t1[:, :], ADD)

            # ---- chunk 0 prologue: mb0 halves first so hp0 scores can
            # start while the mb1 halves are still in flight.
            xcs = {0: (xc0, xc80)}
            qkproj_mb(0, xc80, wq8_sb, None, 0)
            qkproj_mb(0, xc80, wk8_sb, krt, 0)
            # chunk 0's V projection runs here, filling the PE while the DVE
            # RoPE chains for the mb0 halves drain; its vau tiles are only
            # needed once attention starts.
            for tbl in range(4):
                vproj(0, xc0, tbl)
            qkproj_mb(0, xc80, wq8_sb, None, 1)
            qkproj_mb(0, xc80, wk8_sb, krt, 1)


            for sc in range(NCHUNK):
                ntb = 4 * sc + 4
                xc, xc8 = xcs[sc]

                # ---- attention for this chunk, one head-pair at a time,
                # with next-chunk projection, V, W_o and normalize chains
                # woven between attention blocks so no engine sees a burst.
                osb_h = [None, None]
                fulls = list(range(4 * sc))
                diags = [4 * sc + i for i in range(4)]
                if sc == 0:
                    order = diags
                else:
                    k = len(fulls) // 4
                    order = []
                    fi = 0
                    for i, d in enumerate(diags):
                        take = max(2, k) if i == 0 else k
                        take = min(take, len(fulls) - fi)
                        order += fulls[fi:fi + take]
                        fi += take
                        order.append(d)
                    order += fulls[fi:]
                qk_step = max(ntb // 4, 1)

                for hp in range(2):
                    otp = [psO.tile([65, 512], F32, tag="ot", name=f"otp{hi}")
                           for hi in range(2)]
                    blk = hp
                    wo_step = max(ntb // 4, 2)
                    pv_fifo = []  # (pi, tb, lo, es), PV trails scores by 2
                    for pi, tb in enumerate(order):
                        m = tb - 4 * sc
                        lo = 128 * max(m, 0)  # diag: skip cols left of block
                        if hp == 0:
                            if pi == 0 and sc < 3:
                                xcs[sc + 1] = dma_chunk(sc + 1)
                            if pi < 4 and sc > 0:
                                vproj(sc, xc, pi)
                            if pi == 1 and sc == 1:
                                # chunk 0 hp0's bc didn't fit in its 4-block
                                # hp1 loop; runs here
                                emit_bcmult(0, 0, *osb_prev[(0, 0)])
                            if pi == 2 and pend_norm is not None:
                                emit_ln(pend_norm[1][0], pend_norm[1][1])
                            if pi == 6 and pend_norm is not None:
                                emit_bcmult(pend_norm[0], 1, *pend_norm[1])
                                pend_norm = None
                        else:
                            if pi == 2:
                                # deferred normalize of this chunk's hp=0
                                emit_ln(osb_h[0][0], osb_h[0][1])
                            if pi == 6:
                                emit_bcmult(sc, 0, *osb_h[0])
                            if (sc > 0 and pi >= 1
                                    and (pi - 1) % wo_step == 0
                                    and (pi - 1) // wo_step < 4):
                                # W_o of the previous chunk, one s-block at
                                # a time
                                emit_wo_sbl(sc - 1, (pi - 1) // wo_step)
                            if sc < 3 and pi % qk_step == 0 and pi // qk_step < 4:
                                # next chunk's projection, one (proj, mb)
                                # piece at a time: Q-mb0, K-mb0, Q-mb1, K-mb1
                                j = pi // qk_step
                                w_sb_n = (wq8_sb, wk8_sb)[j % 2]
                                dst_n = (None, krt)[j % 2]
                                qkproj_mb(sc + 1, xcs[sc + 1][1], w_sb_n,
                                          dst_n, j // 2)
                        pss = psB.tile([128, 1024], F32, tag="big", name="pss")
                        ps3 = pss[:].rearrange("p (h s) -> p h s", h=2)
                        for hi in range(2):
                            nc.tensor.matmul(
                                ps3[:, hi, lo:512],
                                krt[:, blk, tb * 128:(tb + 1) * 128],
                                qz[:, hi, blk, sc * 512 + lo:(sc + 1) * 512],
                                start=True, stop=True,
                            )
                        es = ep.tile([128, 1024], DT, tag="es", name="es")
                        es3 = es[:].rearrange("p (h s) -> p h s", h=2)
                        with nc.allow_low_precision(reason="rounded matmul input"):
                            nc.scalar.activation(es3[:, :, lo:512], ps3[:, :, lo:512],
                                                 Exp, bias=0.0, scale=SCALE_QK)
                        if m >= 0:  # mask the diagonal 128-col triangle
                            trib = tri_sb[:].rearrange("p (o s) -> p o s", o=1).to_broadcast((128, 2, 128))
                            with nc.allow_low_precision(reason="rounded matmul input"):
                                nc.vector.tensor_tensor(
                                    es3[:, :, lo:lo + 128], es3[:, :, lo:lo + 128],
                                    trib, MUL)
                        # PV trails the score stream by two blocks: the PE
                        # always has scores queued ahead of a PV that may
                        # still be waiting on its exp.
                        pv_fifo.append((pi, tb, lo, es))
                        if len(pv_fifo) > 2:
                            ppi, ptb, plo, pes = pv_fifo.pop(0)
                            for hi in range(2):
                                nc.tensor.matmul(
                                    otp[hi][:, plo:512],
                                    vau[:, ptb, 2 * hp + hi, :],
                                    pes[:, hi * 512 + plo:(hi + 1) * 512],
                                    start=(ppi == 0), stop=False,
                                    skip_group_check=True,
                                )
                    for qi, (ppi, ptb, plo, pes) in enumerate(pv_fifo):
                        for hi in range(2):
                            nc.tensor.matmul(
                                otp[hi][:, plo:512], vau[:, ptb, 2 * hp + hi, :],
                                pes[:, hi * 512 + plo:(hi + 1) * 512],
                                start=(ppi == 0), stop=(qi == len(pv_fifo) - 1),
                                skip_group_check=True,
                            )
                    if sc == 3 and hp == 1:
                        # tail: start the ln chain straight off PSUM so it
                        # overlaps the accumulator drain
                        emit_ln_psum(otp, lnr_sh)
                    # drain O accumulators to SBUF promptly so the two PSUM
                    # banks recycle for the next head-pair; the two copies
                    # run on different engines so the drain latency halves.
                    osb = [op_.tile([65, 512], DT, tag="osb", name=f"osb{hi}")
                           for hi in range(2)]
                    with nc.allow_low_precision(reason="pre-normalize O"):
                        nc.vector.tensor_copy(osb[0][:], otp[0][:])
                        nc.scalar.copy(osb[1][:], otp[1][:])
                    osb_h[hp] = (osb, lnr_sh)

                osb_prev[(sc, 0)] = osb_h[0]
                if sc == 3:
                    # ln already emitted off PSUM at the drain above; heats
                    # keep the PE fed while the ACT chain completes.
                    heat(n=20)
                    emit_bcmult(3, 1, osb_h[1][0], lnr_sh)
                    emit_wo(3)
                else:
                    pend_norm = (sc, osb_h[1])

    if legalize:
        _legalize_waits(nc)
    return nc


def _legalize_waits(nc, max_waits=1):
    """Split >max_waits sync waits onto preceding same-engine NoOps
    (several instruction encodings only have one sync-wait slot)."""
    for fn in nc.m.functions:
        for bb in fn.blocks:
            new_insts = []
            for inst in bb.instructions:
                si = inst.sync_info
                waits = list(si.on_wait) if si is not None and si.on_wait else []
                if len(waits) > max_waits:
                    carry, keep = waits[:-max_waits], waits[-max_waits:]
                    for i, w in enumerate(carry):
                        new_insts.append(mybir.InstNoOp(
                            name=f"{inst.name}_wsplit{i}",
                            engine=inst.engine,
                            bass_nofuse=True,
                            sync_info=mybir.SyncInfo(on_wait=[w], on_update=[]),
                        ))
                    si.on_wait = keep
                new_insts.append(inst)
            bb.instructions[:] = new_insts


def _host_constants():
    # RoPE channel permutation: row r (within a head, 0..63) holds source
    # channel d = 2*i + odd with i = 16*(r//32) + r%16, odd = (r%32)//16.
    r = np.arange(64)
    i_ = 16 * (r // 32) + (r % 16)
    odd = (r % 32) // 16
    dsrc = 2 * i_ + odd  # source channel per permuted row

    inv_freq = ROPE_BASE ** (-(i_.astype(np.float64)) * 2.0 / Dh)
    ang = np.arange(S, dtype=np.float64)[None, :] * inv_freq[:, None]  # [64, S]
    cos64 = np.cos(ang)
    sin64 = np.sin(ang) * np.where(odd == 0, -1.0, 1.0)[:, None]
    cosd = np.tile(cos64, (2, 1)).astype(DT_NP)
    sins = np.tile(sin64, (2, 1)).astype(DT_NP)

    t = np.arange(128)[:, None]
    s = np.arange(128)[None, :]
    trim = (t <= s).astype(DT_NP)

    sel2 = np.zeros((33, 128), np.float32)
    sel2[0, 0:64] = 1
    sel2[32, 64:128] = 1
    return dsrc, cosd, sins, trim, sel2


def _wlay(w, dt=None):  # [E, HD] -> [p, ko, m] contiguous
    return np.ascontiguousarray(w.reshape(8, 128, HD).transpose(1, 0, 2)).astype(dt or DT_NP)


def _wolay(w):  # [HD, E] -> [p, ko, e] contiguous
    return np.ascontiguousarray(w.reshape(2, 128, E).transpose(1, 0, 2)).astype(DT_NP)


_CACHE = {}


def _run(inputs, trace=False):
    if "nc" not in _CACHE:
        _CACHE["nc"] = _build_program()
        _CACHE["consts"] = _host_constants()
    nc = _CACHE["nc"]
    dsrc, cosd, sins, trim, sel2 = _CACHE["consts"]

    x = np.ascontiguousarray(np.asarray(inputs["x"]), dtype=np.float32)
    W_q = np.asarray(inputs["W_q"], dtype=np.float32)
    W_k = np.asarray(inputs["W_k"], dtype=np.float32)
    W_v = np.asarray(inputs["W_v"], dtype=np.float32)
    W_o = np.asarray(inputs["W_o"], dtype=np.float32)

    # [p, sc, eo, s] so each chunk DMA is contiguous per partition
    xTc = [np.ascontiguousarray(
        x[b].reshape(NCHUNK, 512, 8, 128).transpose(3, 0, 2, 1))
        for b in range(B)]
    xTb = [t.astype(DT_NP) for t in xTc]
    xT8 = [t.astype(F8_NP) for t in xTc]

    in_maps = []
    for c in range(8):
        b, g = divmod(c, 4)
        heads = np.arange(4 * g, 4 * g + 4)
        rows_qk = (heads[:, None] * 64 + dsrc[None, :]).reshape(-1)   # permuted
        rows_v = (heads[:, None] * 64 + np.arange(64)[None, :]).reshape(-1)
        in_maps.append({
            "xT": xTb[b],
            "xT8": xT8[b],
            "wq8": _wlay(W_q[rows_qk].T * WS, F8_NP),
            "wk8": _wlay(W_k[rows_qk].T * WS, F8_NP),
            "wv": _wlay(W_v[rows_v].T),
            "wo": _wolay(W_o[:, rows_v].T),
            "cosd": cosd, "sins": sins, "trim": trim, "sel2c": sel2,
            "lnz": np.zeros((33, 512), np.float32),
        })

    res = bass_utils.run_bass_kernel_spmd(
        nc, in_maps, core_ids=list(range(8)), trace=trace,
    )
    out = np.zeros((B, S, E), np.float32)
    for c in range(8):
        out[c // 4] += res.results[c]["y"].astype(np.float32)
    return out, res


def kernel(**inputs):
    out, _ = _run(inputs, trace=False)
    return out
